# revision 1
# baseline (speedup 1.0000x reference)
"""Trainium2 Bass kernel for nn_Block_86672440033530 (sparse_attention).

Transformer block: masked self-attention + AddNorm, class-vector cross-attn
(collapses to a broadcast row since Sk=1) + AddNorm, FFN + AddNorm.

Sharding: 8 cores = 2 batches x 4 query-blocks of 512 rows. Each core
computes full K/V projections for its batch (replicated within the 4-core
batch group), attention for its 512 query rows over all 2048 keys and all 16
heads, then output-projection / LayerNorms / FFN for its rows only. No
cross-core communication; host gathers the 8 row-blocks.

All matmuls run in bf16 (fp32 PSUM accumulation); norms/softmax plumbing in
fp32. The softmax is computed as exp(S)*mask with the denominator taken from
a ones-column appended to V (fused into the AV matmul) and divided out during
PSUM eviction via a DRAM-bounce partition broadcast.
"""
import contextlib
import ctypes
import sys
import types

import numpy as np

if "/opt/trn_rl_repo" not in sys.path:
    sys.path.insert(0, "/opt/trn_rl_repo")

import ml_dtypes  # noqa: E402
import concourse.bass as bass  # noqa: E402
import concourse.mybir as mybir  # noqa: E402
import concourse.tile as tile  # noqa: E402
from concourse.bass_utils import run_bass_kernel_spmd  # noqa: E402
from concourse.masks import make_identity  # noqa: E402

BF16 = mybir.dt.bfloat16
F32 = mybir.dt.float32
NP_BF16 = ml_dtypes.bfloat16

B, S, D, H, DFF = 2, 2048, 1024, 16, 4096
HD = D // H                      # 64
SCALE = float(1.0 / np.sqrt(np.float32(HD)))
NCORES = 8
QS = S // (NCORES // B)          # 512 query rows per core
QT = QS // 128                   # 4 query tiles per core
DT = D // 128                    # 8 d-blocks
ST = S // 128                    # 16 key tiles
FT = DFF // 128                  # 32 dff tiles
EPS = 1e-5


def _install_ntff_shim():
    """The axon image lacks antenv.axon_hooks; register the NTFF profile hook
    via ctypes so run_bass_kernel_spmd(trace=True) works. Harmless if unused."""
    try:
        import antenv
    except ImportError:
        return
    if "antenv.axon_hooks" in sys.modules:
        return

    def _make_hook(so_path):
        try:
            lib = ctypes.CDLL(so_path)
        except OSError:
            return None
        if not hasattr(lib, "axon_start_nrt_profile"):
            return None
        lib.axon_start_nrt_profile.argtypes = [
            ctypes.POINTER(ctypes.c_int64),
            ctypes.c_size_t,
        ]
        lib.axon_start_nrt_profile.restype = ctypes.c_int64
        lib.axon_stop_nrt_profile.argtypes = [ctypes.c_char_p]
        lib.axon_stop_nrt_profile.restype = ctypes.c_int64

        @contextlib.contextmanager
        def _hook(output_dir, device_ids):
            import jax

            jax.devices()
            if device_ids:
                ids = (ctypes.c_int64 * len(device_ids))(*device_ids)
                rc = lib.axon_start_nrt_profile(ids, len(device_ids))
            else:
                rc = lib.axon_start_nrt_profile(None, 0)
            if rc != 0:
                raise RuntimeError(f"axon_start_nrt_profile rc={rc}")
            try:
                yield
            finally:
                n = lib.axon_stop_nrt_profile(str(output_dir).encode())
                print(f"profile: {n} file(s) -> {output_dir}", file=sys.stderr)

        return _hook

    m = types.ModuleType("antenv.axon_hooks")
    m._hook = _make_hook("/opt/axon/libaxon_pjrt.so")
    m.set_axon_ntff_profile_hook = lambda h: setattr(m, "_hook", h)
    m.get_axon_ntff_profile_hook = lambda: m._hook
    sys.modules["antenv.axon_hooks"] = m
    import antenv

    antenv.axon_hooks = m


_install_ntff_shim()


def _split_sync_waits(nc, limit=1):
    """This walrus build accepts at most one sync-wait command per
    instruction; move excess waits onto same-engine NoOps placed before."""
    for func in nc.m.functions:
        for bb in func.blocks:
            out = []
            for ins in bb.instructions:
                si = getattr(ins, "sync_info", None)
                waits = list(si.on_wait) if (si is not None and si.on_wait) else []
                if len(waits) > limit:
                    keep, move = waits[:limit], waits[limit:]
                    for i in range(0, len(move), limit):
                        out.append(
                            mybir.InstNoOp(
                                name=f"{ins.name}-wsplit{i}",
                                sync_info=mybir.SyncInfo(
                                    on_wait=move[i : i + limit], on_update=[]
                                ),
                                bass_nofuse=True,
                                engine=ins.engine,
                            )
                        )
                    si.on_wait = keep
                out.append(ins)
            bb.instructions[:] = out


# ----------------------------------------------------------------------------
# device program (SPMD; identical on all 8 cores, per-core data differs)
# ----------------------------------------------------------------------------

def _build_program():
    nc = bass.Bass()

    def din(name, shape, dt):
        return nc.dram_tensor(name, list(shape), dt, kind="ExternalInput")

    # per-core tensors
    xT = din("xT", [128, DT, S], BF16)          # x[b].T  (d-major)
    xqT = din("xqT", [128, DT, QS], BF16)       # own q rows of xT
    xrows = din("xrows", [QS, D], F32)          # own q rows, natural (residual)
    maskT = din("maskT", [128, ST, QS], BF16)   # mask.T own q cols, tile-major
    cvT = din("cvT", [10, 1], BF16)             # classVector[b].T
    # weights (bf16, shared; wq/bq pre-scaled by 1/sqrt(hd))
    wq = din("wq", [D, D], BF16)
    wk = din("wk", [D, D], BF16)
    wv = din("wv", [D, D], BF16)
    wo = din("wo", [D, D], BF16)
    w1 = din("w1", [D, DFF], BF16)
    w2 = din("w2", [DFF, D], BF16)
    cew = din("cew", [10, D], BF16)
    cawv = din("cawv", [D, D], BF16)
    cawo = din("cawo", [D, D], BF16)
    # f32 bias/ln vectors: column-interleaved [128, n] or rows [1, n]
    bq_c = din("bq_c", [128, DT], F32)
    bk_c = din("bk_c", [128, DT], F32)
    b1_c = din("b1_c", [128, FT], F32)
    ceb_c = din("ceb_c", [128, DT], F32)
    cabv_c = din("cabv_c", [128, DT], F32)
    cabo_c = din("cabo_c", [128, DT], F32)
    bv_r = din("bv_r", [1, D], F32)
    bo_r = din("bo_r", [1, D], F32)
    b2_r = din("b2_r", [1, D], F32)
    g1_r = din("g1_r", [1, D], F32)
    lb1_r = din("lb1_r", [1, D], F32)
    g2_r = din("g2_r", [1, D], F32)
    lb2_r = din("lb2_r", [1, D], F32)
    g3_r = din("g3_r", [1, D], F32)
    lb3_r = din("lb3_r", [1, D], F32)

    out_d = nc.dram_tensor("out", [QS, D], F32, kind="ExternalOutput")

    Exp = mybir.ActivationFunctionType.Exp
    Relu = mybir.ActivationFunctionType.Relu
    Sqrt = mybir.ActivationFunctionType.Sqrt
    ADD = mybir.AluOpType.add
    SUB = mybir.AluOpType.subtract
    MUL = mybir.AluOpType.mult

    with tile.TileContext(nc) as tc, contextlib.ExitStack() as ctx:
        # -------- whole-kernel residents (small) ---------------------------
        res = ctx.enter_context(tc.tile_pool(name="res", bufs=1))
        dres = ctx.enter_context(tc.tile_pool(name="dres", bufs=1, space="DRAM"))

        ident = res.tile([128, 128], BF16)
        make_identity(nc, ident)
        eps_t = res.tile([128, 1], F32)
        nc.vector.memset(eps_t[:], EPS)
        oT_s = res.tile([128, DT, QS], BF16)     # attention output (transposed)
        r_b = res.tile([128, D], F32)            # cross-attn row, broadcast

        def bcast_load(pool, src_row, n, tag):
            t = pool.tile([128, n], F32, tag=tag)
            nc.sync.dma_start(out=t[:], in_=src_row[0:1, :].broadcast_to((128, n)))
            return t

        def layer_norm(pool, dst, src, g_b=None, lb_b=None):
            """dst = LN_freedim(src) [* g] [+ b] for [128, D] f32 views."""
            stats = pool.tile([128, 2, 6], F32, tag="lnst")
            mv = pool.tile([128, 2], F32, tag="lnmv")
            for sg in range(2):
                nc.vector.bn_stats(
                    out=stats[:, sg, :], in_=src[:, sg * 512 : (sg + 1) * 512]
                )
            nc.vector.bn_aggr(out=mv[:], in_=stats[:])
            rstd = pool.tile([128, 1], F32, tag="lnrs")
            nc.scalar.activation(
                out=rstd[:], in_=mv[:, 1:2], func=Sqrt, bias=eps_t[:]
            )
            nc.vector.reciprocal(out=rstd[:], in_=rstd[:])
            nc.vector.tensor_scalar(
                out=dst[:], in0=src[:], scalar1=mv[:, 0:1], scalar2=rstd[:],
                op0=SUB, op1=MUL,
            )
            if g_b is not None:
                nc.vector.tensor_mul(out=dst[:], in0=dst[:], in1=g_b[:])
            if lb_b is not None:
                nc.vector.tensor_add(out=dst[:], in0=dst[:], in1=lb_b[:])

        # -------- phases P+A share the big attention residents -------------
        with tc.tile_pool(name="pa", bufs=1) as pa:
            kT_s = pa.tile([128, DT, S], BF16)          # K.T (d-major), +bk
            vp_s = pa.tile([128, ST, H, HD + 1], BF16)  # V natural + ones col
            qT_s = pa.tile([128, H, QS], BF16)          # Q.T zero-padded to K=128
            maskT_s = pa.tile([128, ST * QS], BF16)
            xqT_s = pa.tile([128, DT, QS], BF16)

            # ---- phase P: K/V/Q projections -------------------------------
            with tc.tile_pool(name="pph", bufs=1) as pp, \
                 tc.tile_pool(name="pw", bufs=12) as pw, \
                 tc.tile_pool(name="rph", bufs=1) as rp, \
                 tc.tile_pool(name="pps", bufs=5, space="PSUM") as pps:
                xT_s = pp.tile([128, DT, S], BF16)
                for k in range(DT):
                    nc.sync.dma_start(out=xT_s[:, k, :], in_=xT[:, k, :])
                bq_s = pp.tile([128, DT], F32)
                bk_s = pp.tile([128, DT], F32)
                nc.sync.dma_start(out=bk_s[:], in_=bk_c[:])
                nc.sync.dma_start(out=bq_s[:], in_=bq_c[:])
                bv_b = bcast_load(pp, bv_r, D, "bvb")

                nc.vector.memset(vp_s[:, :, :, HD : HD + 1], 1.0)

                # K: kT[:, m, ns] = wk[:, m].T @ xT  (+bk)
                wk_t = [pw.tile([128, D], BF16, tag="w", name=f"wk_t{_k}") for _k in range(DT)]
                for k in range(DT):
                    nc.gpsimd.dma_start(
                        out=wk_t[k][:], in_=wk[k * 128 : (k + 1) * 128, :]
                    )
                cv_bf = rp.tile([10, 1], BF16)
                cew_s = rp.tile([10, D], BF16)
                ceb_s = rp.tile([128, DT], F32)
                cabv_s = rp.tile([128, DT], F32)
                cabo_s = rp.tile([128, DT], F32)
                cawv_blk = rp.tile([128, DT, D], BF16, tag="rwblk")
                cawo_blk = rp.tile([128, DT, D], BF16, tag="rwblk", name="cawo_blk")
                cvec = rp.tile([128, DT], BF16)
                vcv = rp.tile([128, DT], BF16)
                rT = rp.tile([128, DT], F32)

                def r_stage1():
                    cv_ps = pps.tile([128, DT], F32, tag="rp1", bufs=1)
                    for m in range(DT):
                        nc.tensor.matmul(
                            cv_ps[:, m : m + 1],
                            cew_s[:, m * 128 : (m + 1) * 128],
                            cv_bf[:], start=True, stop=True,
                        )
                    nc.vector.tensor_add(out=cvec[:], in0=cv_ps[:], in1=ceb_s[:])

                def r_stage2():
                    vcv_ps = pps.tile([128, DT], F32, tag="rp2", bufs=1)
                    for m in range(DT):
                        for k in range(DT):
                            nc.tensor.matmul(
                                vcv_ps[:, m : m + 1],
                                cawv_blk[:, k, m * 128 : (m + 1) * 128],
                                cvec[:, k : k + 1],
                                start=(k == 0), stop=(k == DT - 1),
                            )
                    nc.vector.tensor_add(out=vcv[:], in0=vcv_ps[:], in1=cabv_s[:])

                def r_stage3():
                    r_ps = pps.tile([128, DT], F32, tag="rp3", bufs=1)
                    for m in range(DT):
                        for k in range(DT):
                            nc.tensor.matmul(
                                r_ps[:, m : m + 1],
                                cawo_blk[:, k, m * 128 : (m + 1) * 128],
                                vcv[:, k : k + 1],
                                start=(k == 0), stop=(k == DT - 1),
                            )
                    nc.vector.tensor_add(out=rT[:], in0=r_ps[:], in1=cabo_s[:])
                    r_dram = dres.tile([D], F32)
                    nc.sync.dma_start(
                        out=r_dram.rearrange("(a p) -> p a", p=128), in_=rT[:]
                    )
                    nc.sync.dma_start(
                        out=r_b[:], in_=r_dram[None, :].broadcast_to((128, D))
                    )

                for m in range(DT):
                    if m == 1:
                        nc.sync.dma_start(out=cv_bf[:], in_=cvT[:])
                        nc.sync.dma_start(out=cew_s[:], in_=cew[:])
                        nc.sync.dma_start(out=ceb_s[:], in_=ceb_c[:])
                        nc.sync.dma_start(out=cabv_s[:], in_=cabv_c[:])
                        nc.sync.dma_start(out=cabo_s[:], in_=cabo_c[:])
                        nc.gpsimd.dma_start(
                            out=cawv_blk[:],
                            in_=cawv.rearrange("(a p) n -> p a n", p=128),
                        )
                    if m == 4:
                        r_stage1()
                    pss = [
                        pps.tile([128, 512], F32, tag="pj", name=f"kps{m}_{ns}")
                        for ns in range(S // 512)
                    ]
                    for k in range(DT):
                        for ns in range(S // 512):
                            nc.tensor.matmul(
                                pss[ns][:],
                                wk_t[k][:, m * 128 : (m + 1) * 128],
                                xT_s[:, k, ns * 512 : (ns + 1) * 512],
                                start=(k == 0), stop=(k == DT - 1),
                            )
                    for ns in range(S // 512):
                        nc.vector.tensor_scalar(
                            out=kT_s[:, m, ns * 512 : (ns + 1) * 512],
                            in0=pss[ns][:], scalar1=bk_s[:, m : m + 1],
                            scalar2=None, op0=ADD,
                        )

                r_stage2()
                nc.gpsimd.dma_start(
                    out=cawo_blk[:], in_=cawo.rearrange("(a p) n -> p a n", p=128)
                )
                nc.sync.dma_start(out=xqT_s[:], in_=xqT[:])
                # Q (own rows), zero-padded per head so QK runs at K=128:
                # even head data in partitions 0-63, odd in 64-127.
                nc.vector.memset(qT_s[64:128, 0:H:2, :], 0.0)
                nc.vector.memset(qT_s[0:64, 1:H:2, :], 0.0)
                wq_t = [pw.tile([128, D], BF16, tag="w", name=f"wq_t{_k}") for _k in range(DT)]
                for k in range(DT):
                    nc.gpsimd.dma_start(
                        out=wq_t[k][:], in_=wq[k * 128 : (k + 1) * 128, :]
                    )
                for m in range(DT):
                    ps = pps.tile([128, QS], F32, tag="pj")
                    for k in range(DT):
                        nc.tensor.matmul(
                            ps[:],
                            wq_t[k][:, m * 128 : (m + 1) * 128],
                            xqT_s[:, k, :],
                            start=(k == 0), stop=(k == DT - 1),
                        )
                    nc.vector.tensor_scalar(
                        out=qT_s[0:64, 2 * m, :], in0=ps[0:64, :],
                        scalar1=bq_s[0:64, m : m + 1], scalar2=None, op0=ADD,
                    )
                    nc.vector.tensor_scalar(
                        out=qT_s[64:128, 2 * m + 1, :], in0=ps[64:128, :],
                        scalar1=bq_s[64:128, m : m + 1], scalar2=None, op0=ADD,
                    )

                r_stage3()
                # V (natural): vp[:, st, heads, :64] = xT[:, :, st].T @ wv (+bv)
                wv_t = [pw.tile([128, D], BF16, tag="w", name=f"wv_t{_k}") for _k in range(DT)]
                for k in range(DT):
                    nc.gpsimd.dma_start(
                        out=wv_t[k][:], in_=wv[k * 128 : (k + 1) * 128, :]
                    )
                for st in range(ST):
                    if st == 2:
                        nc.sync.dma_start(
                            out=maskT_s[:],
                            in_=maskT.rearrange("p a q -> p (a q)"),
                        )
                    pss = [
                        pps.tile([128, 512], F32, tag="pj", name=f"vps{st}_{c}")
                        for c in range(D // 512)
                    ]
                    for k in range(DT):
                        for c in range(D // 512):
                            nc.tensor.matmul(
                                pss[c][:],
                                xT_s[:, k, st * 128 : (st + 1) * 128],
                                wv_t[k][:, c * 512 : (c + 1) * 512],
                                start=(k == 0), stop=(k == DT - 1),
                            )
                    for c in range(D // 512):
                        nc.vector.tensor_add(
                            out=vp_s[:, st, c * 8 : (c + 1) * 8, 0:HD],
                            in0=pss[c][:].rearrange("p (h e) -> p h e", e=HD),
                            in1=bv_b[:, c * 512 : (c + 1) * 512].rearrange(
                                "p (h e) -> p h e", e=HD
                            ),
                        )

            # ---- phase A: attention ---------------------------------------
            with tc.tile_pool(name="aph", bufs=4) as apl, \
                 tc.tile_pool(name="aps", bufs=3, space="PSUM") as aps, \
                 tc.tile_pool(name="avps", bufs=2, space="PSUM") as avps, \
                 tc.tile_pool(name="adr", bufs=1, space="DRAM") as adr:
                den_d = [
                    adr.tile([4, QS], F32, tag="dend", name=f"den_d{_b}", bufs=4)
                    for _b in range(4)
                ]
                den_d2 = [
                    adr.tile([4, QS], F32, tag="dend2", name=f"den_d2{_b}", bufs=4)
                    for _b in range(4)
                ]

                def normalize_batch(b, tail=False):
                    # reciprocal of this batch's 4 den rows ([32,64] shape for
                    # lane parallelism), then broadcast and scale 2 oT tiles
                    den_sb = apl.tile([32, 64], F32, tag="densb", name=f"densb{b}")
                    flat = den_d[b].rearrange("a q -> (a q)")
                    nc.sync.dma_start(
                        out=den_sb[:], in_=flat.rearrange("(p f) -> p f", f=64)
                    )
                    nc.vector.reciprocal(out=den_sb[:], in_=den_sb[:])
                    flat2 = den_d2[b].rearrange("a q -> (a q)")
                    nc.sync.dma_start(
                        out=flat2.rearrange("(p f) -> p f", f=64), in_=den_sb[:]
                    )
                    rb2 = apl.tile([128, 2, QS], F32, tag="rb2", name=f"rb2{b}")
                    dv = den_d2[b].rearrange("(a e) q -> e a q", e=2)
                    nc.sync.dma_start(
                        out=rb2[0:64, :, :],
                        in_=dv[0:1, :, :].broadcast_to((64, 2, QS)),
                    )
                    nc.sync.dma_start(
                        out=rb2[64:128, :, :],
                        in_=dv[1:2, :, :].broadcast_to((64, 2, QS)),
                    )
                    eng = nc.vector if tail else nc.gpsimd
                    for tt in range(2):
                        t = b * 2 + tt
                        eng.tensor_mul(
                            out=oT_s[:, t, :], in0=oT_s[:, t, :],
                            in1=rb2[:, tt, :],
                        )

                for h in range(H):
                    pb = (h % 2) * 64
                    dtile = h // 2
                    av = avps.tile([HD + 1, QS], F32, tag="av")
                    for jp in range(ST // 2):
                        qk = aps.tile([128, 2, 512], F32, tag="qk")
                        for hf in range(2):
                            j = jp * 2 + hf
                            nc.tensor.matmul(
                                qk[:, hf, :],
                                kT_s[:, dtile, j * 128 : (j + 1) * 128],
                                qT_s[:, h, :],
                                start=True, stop=True,
                            )
                        pe_t = apl.tile([128, 1024], BF16, tag="pe_t")
                        nc.scalar.activation(pe_t[:], qk[:], Exp)
                        pt = apl.tile([128, 1024], BF16, tag="pt")
                        nc.vector.tensor_mul(
                            out=pt[:], in0=pe_t[:],
                            in1=maskT_s[:, jp * 1024 : (jp + 1) * 1024],
                        )
                        for hf in range(2):
                            j = jp * 2 + hf
                            nc.tensor.matmul(
                                av[:],
                                vp_s[:, j, h, :],
                                pt[:, hf * 512 : (hf + 1) * 512],
                                start=(j == 0), stop=(j == ST - 1),
                            )
                    # stash denominator row (via DRAM); evict unnormalized
                    dr_row = apl.tile([1, QS], F32, tag="dr", bufs=2)
                    nc.vector.tensor_copy(out=dr_row[:], in_=av[HD : HD + 1, :])
                    nc.sync.dma_start(
                        out=den_d[h // 4][h % 4 : h % 4 + 1, :], in_=dr_row[:]
                    )
                    nc.vector.tensor_copy(
                        out=oT_s[pb : pb + 64, dtile, :], in_=av[0:HD, :]
                    )
                    if h == 7:
                        normalize_batch(0)
                    elif h == 11:
                        normalize_batch(1)
                    elif h == 14:
                        normalize_batch(2)
                normalize_batch(3, tail=True)

        # -------- phase O: out-proj, AddNorm, cross-attn row, AddNorm ------
        with tc.tile_pool(name="of", bufs=1) as of:
            h2_s = of.tile([128, QT, D], F32)
            h2T_s = of.tile([128, DT, QS], BF16)
            with tc.tile_pool(name="oph", bufs=1) as op, \
                 tc.tile_pool(name="ow", bufs=9) as ow, \
                 tc.tile_pool(name="ops", bufs=4, space="PSUM") as ops, \
                 tc.tile_pool(name="otps", bufs=2, space="PSUM") as otps, \
                 tc.tile_pool(name="oln", bufs=4) as oln:
                xr_s = op.tile([128, QT, D], F32)
                nc.sync.dma_start(
                    out=xr_s[:], in_=xrows.rearrange("(t p) d -> p t d", p=128)
                )
                bo_b = bcast_load(op, bo_r, D, "bob")
                g1_b = bcast_load(op, g1_r, D, "g1b")
                lb1_b = bcast_load(op, lb1_r, D, "lb1b")
                # rc = lb1 + r : single chain add in LN1->LN2 handoff
                nc.vector.tensor_add(out=lb1_b[:], in0=lb1_b[:], in1=r_b[:])

                wo_t = [ow.tile([128, D], BF16, tag="wo", name=f"wo_t{_k}") for _k in range(DT)]
                for k in range(DT):
                    nc.gpsimd.dma_start(
                        out=wo_t[k][:], in_=wo[k * 128 : (k + 1) * 128, :]
                    )

                h_s = op.tile([128, QT, D], F32)
                for mq in range(QT):
                    nc.vector.tensor_add(
                        out=xr_s[:, mq, :], in0=xr_s[:, mq, :], in1=bo_b[:]
                    )
                for mq in range(QT):
                    pss = [
                        ops.tile([128, 512], F32, tag="op", name=f"ops{mq}_{ns}")
                        for ns in range(D // 512)
                    ]
                    for k in range(DT):
                        for ns in range(D // 512):
                            nc.tensor.matmul(
                                pss[ns][:],
                                oT_s[:, k, mq * 128 : (mq + 1) * 128],
                                wo_t[k][:, ns * 512 : (ns + 1) * 512],
                                start=(k == 0), stop=(k == DT - 1),
                            )
                    for ns in range(D // 512):
                        sl = slice(ns * 512, (ns + 1) * 512)
                        nc.vector.tensor_add(
                            out=h_s[:, mq, sl], in0=pss[ns][:], in1=xr_s[:, mq, sl]
                        )
                    layer_norm(oln, h_s[:, mq, :], h_s[:, mq, :], g1_b, lb1_b)
                    # h2n = pure-normalized LN2; gamma2/beta2 folded into w1/b1
                    # (host) and into the FF2-tail residual
                    layer_norm(oln, h2_s[:, mq, :], h_s[:, mq, :])
                    h2bf = oln.tile([128, D], BF16, tag="h2bf")
                    nc.scalar.copy(out=h2bf[:], in_=h2_s[:, mq, :])
                    for t in range(DT):
                        tp = otps.tile([128, 128], BF16, tag="tp")
                        nc.tensor.transpose(
                            tp[:], h2bf[:, t * 128 : (t + 1) * 128], ident[:]
                        )
                        nc.scalar.copy(
                            out=h2T_s[:, t, mq * 128 : (mq + 1) * 128], in_=tp[:]
                        )

            # -------- phase F: FFN + AddNorm -------------------------------
            with tc.tile_pool(name="fph", bufs=1) as fp, \
                 tc.tile_pool(name="fln", bufs=4) as fln:
                b1_s = fp.tile([128, FT], F32)
                nc.sync.dma_start(out=b1_s[:], in_=b1_c[:])
                g2_b = bcast_load(fp, g2_r, D, "g2b")
                cb_b = bcast_load(fp, b2_r, D, "cbb")   # lb2 + b2 (host)
                g3_b = bcast_load(fp, g3_r, D, "g3b")
                lb3_b = bcast_load(fp, lb3_r, D, "lb3b")
                # h2full = h2n * g2 + (lb2 + b2), off the critical path
                for mq in range(QT):
                    nc.gpsimd.tensor_mul(
                        out=h2_s[:, mq, :], in0=h2_s[:, mq, :], in1=g2_b[:]
                    )
                    nc.gpsimd.tensor_add(
                        out=h2_s[:, mq, :], in0=h2_s[:, mq, :], in1=cb_b[:]
                    )

                fT_s = fp.tile([128, FT, QS], BF16)
                # FF1: fT[:, mf, :] = relu(w1[:, mf].T @ h2T + b1)
                with tc.tile_pool(name="fw1", bufs=2) as fw1, \
                     tc.tile_pool(name="fps", bufs=3, space="PSUM") as fps:
                    for mfg in range(4):
                        w1_t = fw1.tile([128, DT, 1024], BF16, tag="w1")
                        nc.gpsimd.dma_start(
                            out=w1_t[:],
                            in_=w1.rearrange("(a p) n -> p a n", p=128)[
                                :, :, mfg * 1024 : (mfg + 1) * 1024
                            ],
                        )
                        for mfl in range(8):
                            mf = mfg * 8 + mfl
                            ps = fps.tile([128, QS], F32, tag="f1")
                            for k in range(DT):
                                nc.tensor.matmul(
                                    ps[:],
                                    w1_t[:, k, mfl * 128 : (mfl + 1) * 128],
                                    h2T_s[:, k, :],
                                    start=(k == 0), stop=(k == DT - 1),
                                )
                            nc.scalar.activation(
                                out=fT_s[:, mf, :], in_=ps[:], func=Relu,
                                bias=b1_s[:, mf : mf + 1],
                            )

                # FF2 in two mq-halves so the first half's LN3/output
                # overlaps the second half's matmuls
                out_t = fp.tile([128, QT, D], F32)
                with tc.tile_pool(name="fw2", bufs=3) as fw2, \
                     tc.tile_pool(name="f2ps", bufs=8, space="PSUM") as f2ps:
                    for half in range(2):
                        mqs = (0, 1, 2) if half == 0 else (3,)
                        ps2 = [
                            f2ps.tile([128, 512], F32, tag="f2",
                                      name=f"ps2_{half}_{_i}")
                            for _i in range(2 * len(mqs))
                        ]
                        for kf in range(FT):
                            w2_t = fw2.tile([128, D], BF16, tag="w2",
                                            name=f"w2_t{half}_{kf}")
                            nc.gpsimd.dma_start(
                                out=w2_t[:], in_=w2[kf * 128 : (kf + 1) * 128, :]
                            )
                            for i, mq in enumerate(mqs):
                                for ns in range(D // 512):
                                    nc.tensor.matmul(
                                        ps2[i * 2 + ns][:],
                                        fT_s[:, kf, mq * 128 : (mq + 1) * 128],
                                        w2_t[:, ns * 512 : (ns + 1) * 512],
                                        start=(kf == 0), stop=(kf == FT - 1),
                                    )
                        for i, mq in enumerate(mqs):
                            for ns in range(D // 512):
                                sl = slice(ns * 512, (ns + 1) * 512)
                                nc.vector.tensor_add(
                                    out=out_t[:, mq, sl], in0=ps2[i * 2 + ns][:],
                                    in1=h2_s[:, mq, sl],
                                )
                            layer_norm(
                                fln, out_t[:, mq, :], out_t[:, mq, :], g3_b, lb3_b
                            )
                            nc.sync.dma_start(
                                out=out_d.rearrange("(t p) d -> p t d", p=128)[:, mq, :],
                                in_=out_t[:, mq, :],
                            )

    _split_sync_waits(nc)
    return nc


_NC_CACHE = None


def _get_program():
    global _NC_CACHE
    if _NC_CACHE is None:
        _NC_CACHE = _build_program()
    return _NC_CACHE


# ----------------------------------------------------------------------------
# host wrapper
# ----------------------------------------------------------------------------

def _col_interleave(v, nt):
    """[n] f32 -> [128, nt] where col j holds v[j*128:(j+1)*128]."""
    return np.ascontiguousarray(
        np.asarray(v, np.float32).reshape(nt, 128).T
    )


def kernel(**inputs):
    x = np.asarray(inputs["cur_input"], np.float32)          # [B, S, D]
    cls = np.asarray(inputs["classVector"], np.float32)      # [B, 1, 10]
    mask = np.asarray(inputs["attn_mask"])                   # [S, S] bool

    bf = lambda a: np.ascontiguousarray(np.asarray(a, np.float32)).astype(NP_BF16)
    f32 = lambda a: np.ascontiguousarray(np.asarray(a, np.float32))
    row = lambda v: f32(np.asarray(v, np.float32).reshape(1, -1))

    shared = dict(
        wq=bf(np.asarray(inputs["sa_wq"], np.float32) * SCALE),
        wk=bf(inputs["sa_wk"]),
        wv=bf(inputs["sa_wv"]),
        wo=bf(inputs["sa_wo"]),
        # gamma2 folded into w1 rows; lb2 folded into b1 (and into b2_r below)
        w1=bf(np.asarray(inputs["ff_w1"], np.float32)
              * np.asarray(inputs["ln2_g"], np.float32)[:, None]),
        w2=bf(inputs["ff_w2"]),
        cew=bf(inputs["ce_w"]),
        cawv=bf(inputs["ca_wv"]),
        cawo=bf(inputs["ca_wo"]),
        bq_c=_col_interleave(np.asarray(inputs["sa_bq"], np.float32) * SCALE, DT),
        bk_c=_col_interleave(inputs["sa_bk"], DT),
        b1_c=_col_interleave(
            np.asarray(inputs["ff_b1"], np.float32)
            + np.asarray(inputs["ln2_b"], np.float32)
            @ np.asarray(inputs["ff_w1"], np.float32), FT),
        ceb_c=_col_interleave(inputs["ce_b"], DT),
        cabv_c=_col_interleave(inputs["ca_bv"], DT),
        cabo_c=_col_interleave(inputs["ca_bo"], DT),
        bv_r=row(inputs["sa_bv"]),
        bo_r=row(inputs["sa_bo"]),
        b2_r=row(np.asarray(inputs["ff_b2"], np.float32)
                 + np.asarray(inputs["ln2_b"], np.float32)),
        g1_r=row(inputs["ln1_g"]),
        lb1_r=row(inputs["ln1_b"]),
        g2_r=row(inputs["ln2_g"]),
        lb2_r=row(inputs["ln2_b"]),
        g3_r=row(inputs["ln3_g"]),
        lb3_r=row(inputs["ln3_b"]),
    )

    mT = mask.T.astype(np.float32)  # [S key, S query]
    in_maps = []
    for c in range(NCORES):
        b, q0 = c // (NCORES // B), (c % (NCORES // B)) * QS
        xTb = x[b].T.reshape(DT, 128, S).transpose(1, 0, 2)       # [128, DT, S]
        mTc = mT[:, q0 : q0 + QS].reshape(ST, 128, QS).transpose(1, 0, 2)
        in_maps.append(
            dict(
                shared,
                xT=bf(xTb),
                xqT=bf(xTb[:, :, q0 : q0 + QS]),
                xrows=f32(x[b, q0 : q0 + QS, :]),
                maskT=bf(mTc),
                cvT=bf(cls[b, 0].reshape(10, 1)),
            )
        )

    res = run_bass_kernel_spmd(_get_program(), in_maps, list(range(NCORES)))
    out = np.empty((B, S, D), np.float32)
    for c in range(NCORES):
        b, q0 = c // (NCORES // B), (c % (NCORES // B)) * QS
        out[b, q0 : q0 + QS] = res.results[c]["out"]
    return out



# revision 2
# speedup vs baseline: 1.1740x; 1.1740x over previous
"""Trainium2 Bass kernel for nn_Block_86672440033530 (sparse_attention).

Transformer block: masked self-attention + AddNorm, class-vector cross-attn
(collapses to a host-computed broadcast row since Sk=1) + AddNorm, FFN + AddNorm.

Sharding: 8 cores = 2 batches x 4 query-blocks of 512 rows. K/V projections are
sharded across each 4-core batch group (each core projects its own 512 keys)
and exchanged with two AllGather collectives; everything else is row-local.

Precision: Q/K/V projections, QK, and AV run in fp8(e4m3) — DoubleRow fp8
matmuls for the projections and AV (K=256 per step), row-tiled K=64 matmul
pairs for QK (two heads concurrently in the PE array). The attention output is
a tiny fraction of the residual stream here, so fp8 error washes out. O-proj
and the FFN stay bf16 (they carry ~half the stream); PSUM always fp32. The
softmax denominator comes from a ones-column appended to V (fused into the AV
matmul) and is divided out via a DRAM-bounce partition broadcast.
"""
import contextlib
import ctypes
import sys
import types

import numpy as np

if "/opt/trn_rl_repo" not in sys.path:
    sys.path.insert(0, "/opt/trn_rl_repo")

import ml_dtypes  # noqa: E402
import concourse.bass as bass  # noqa: E402
import concourse.mybir as mybir  # noqa: E402
import concourse.tile as tile  # noqa: E402
from concourse.bass_utils import run_bass_kernel_spmd  # noqa: E402
from concourse.masks import make_identity  # noqa: E402

BF16 = mybir.dt.bfloat16
F32 = mybir.dt.float32
F8 = mybir.dt.float8e4
NP_BF16 = ml_dtypes.bfloat16
NP_F8 = ml_dtypes.float8_e4m3

B, S, D, H, DFF = 2, 2048, 1024, 16, 4096
HD = D // H                      # 64
NCORES = 8
QS = S // (NCORES // B)          # 512 query rows per core
QT = QS // 128                   # 4 query tiles per core
DT = D // 128                    # 8 d-blocks
ST = S // 128                    # 16 key tiles
FT = DFF // 128                  # 32 dff tiles
EPS = 1e-5
GROUPS = [[0, 1, 2, 3], [4, 5, 6, 7]]
DR = mybir.MatmulPerfMode.DoubleRow


def _install_ntff_shim():
    """The axon image lacks antenv.axon_hooks; register the NTFF profile hook
    via ctypes so run_bass_kernel_spmd(trace=True) works. Harmless if unused."""
    try:
        import antenv
    except ImportError:
        return
    if "antenv.axon_hooks" in sys.modules:
        return

    def _make_hook(so_path):
        try:
            lib = ctypes.CDLL(so_path)
        except OSError:
            return None
        if not hasattr(lib, "axon_start_nrt_profile"):
            return None
        lib.axon_start_nrt_profile.argtypes = [
            ctypes.POINTER(ctypes.c_int64),
            ctypes.c_size_t,
        ]
        lib.axon_start_nrt_profile.restype = ctypes.c_int64
        lib.axon_stop_nrt_profile.argtypes = [ctypes.c_char_p]
        lib.axon_stop_nrt_profile.restype = ctypes.c_int64

        @contextlib.contextmanager
        def _hook(output_dir, device_ids):
            import jax

            jax.devices()
            if device_ids:
                ids = (ctypes.c_int64 * len(device_ids))(*device_ids)
                rc = lib.axon_start_nrt_profile(ids, len(device_ids))
            else:
                rc = lib.axon_start_nrt_profile(None, 0)
            if rc != 0:
                raise RuntimeError(f"axon_start_nrt_profile rc={rc}")
            try:
                yield
            finally:
                n = lib.axon_stop_nrt_profile(str(output_dir).encode())
                print(f"profile: {n} file(s) -> {output_dir}", file=sys.stderr)

        return _hook

    m = types.ModuleType("antenv.axon_hooks")
    m._hook = _make_hook("/opt/axon/libaxon_pjrt.so")
    m.set_axon_ntff_profile_hook = lambda h: setattr(m, "_hook", h)
    m.get_axon_ntff_profile_hook = lambda: m._hook
    sys.modules["antenv.axon_hooks"] = m
    import antenv

    antenv.axon_hooks = m


_install_ntff_shim()


def _split_sync_waits(nc, limit=1):
    """This walrus build accepts at most one sync-wait command per
    instruction; move excess waits onto same-engine NoOps placed before."""
    for func in nc.m.functions:
        for bb in func.blocks:
            out = []
            for ins in bb.instructions:
                si = getattr(ins, "sync_info", None)
                waits = list(si.on_wait) if (si is not None and si.on_wait) else []
                if len(waits) > limit:
                    keep, move = waits[:limit], waits[limit:]
                    for i in range(0, len(move), limit):
                        out.append(
                            mybir.InstNoOp(
                                name=f"{ins.name}-wsplit{i}",
                                sync_info=mybir.SyncInfo(
                                    on_wait=move[i : i + limit], on_update=[]
                                ),
                                bass_nofuse=True,
                                engine=ins.engine,
                            )
                        )
                    si.on_wait = keep
                out.append(ins)
            bb.instructions[:] = out


# ----------------------------------------------------------------------------
# device program (SPMD; identical on all 8 cores, per-core data differs)
# ----------------------------------------------------------------------------

def _build_program():
    nc = bass.Bass()

    def din(name, shape, dt):
        return nc.dram_tensor(name, list(shape), dt, kind="ExternalInput")

    # per-core tensors
    xqT8 = din("xqT8", [128, DT, QS], F8)       # own rows of x[b].T, fp8
    xrows = din("xrows", [QS, D], F32)          # own rows, natural (residual)
    maskT = din("maskT", [128, ST, QS], BF16)   # mask.T own q cols {0,1}
    # weights
    wq8 = din("wq8", [D, D], F8)
    wk8 = din("wk8", [D, D], F8)
    wv8 = din("wv8", [D, D], F8)
    wo = din("wo", [D, D], BF16)
    w1 = din("w1", [D, DFF], BF16)              # gamma2 folded into rows
    w2 = din("w2", [DFF, D], BF16)
    # f32 bias/ln vectors: column-interleaved [128, n] or rows [1, n]
    bq_c = din("bq_c", [128, DT], F32)
    bk_c = din("bk_c", [128, DT], F32)
    b1_c = din("b1_c", [128, FT], F32)
    bv_r = din("bv_r", [1, D], F32)
    bo_r = din("bo_r", [1, D], F32)
    cb_r = din("cb_r", [1, D], F32)             # lb2 + b2 (host)
    g1_r = din("g1_r", [1, D], F32)
    lb1r_r = din("lb1r_r", [1, D], F32)         # ln1_b + cross-attn row (host)
    g2_r = din("g2_r", [1, D], F32)
    g3_r = din("g3_r", [1, D], F32)
    lb3_r = din("lb3_r", [1, D], F32)

    out_d = nc.dram_tensor("out", [QS, D], F32, kind="ExternalOutput")

    Exp = mybir.ActivationFunctionType.Exp
    Relu = mybir.ActivationFunctionType.Relu
    Sqrt = mybir.ActivationFunctionType.Sqrt
    ADD = mybir.AluOpType.add
    SUB = mybir.AluOpType.subtract
    MUL = mybir.AluOpType.mult

    with tile.TileContext(nc) as tc, contextlib.ExitStack() as ctx:
        # -------- whole-kernel residents (small) ---------------------------
        res = ctx.enter_context(tc.tile_pool(name="res", bufs=1))

        ident = res.tile([128, 128], BF16)
        make_identity(nc, ident)
        eps_t = res.tile([128, 1], F32)
        nc.vector.memset(eps_t[:], EPS)
        oT_s = res.tile([128, DT, QS], BF16)     # attention output (transposed)

        def bcast_load(pool, src_row, n, tag, eng=None):
            t = pool.tile([128, n], F32, tag=tag)
            e = eng if eng is not None else nc.sync
            e.dma_start(out=t[:], in_=src_row[0:1, :].broadcast_to((128, n)))
            return t

        def layer_norm(pool, dst, src, g_b=None, lb_b=None):
            """dst = LN_freedim(src) [* g] [+ b] for [128, D] f32 views."""
            stats = pool.tile([128, 2, 6], F32, tag="lnst")
            mv = pool.tile([128, 2], F32, tag="lnmv")
            for sg in range(2):
                nc.vector.bn_stats(
                    out=stats[:, sg, :], in_=src[:, sg * 512 : (sg + 1) * 512]
                )
            nc.vector.bn_aggr(out=mv[:], in_=stats[:])
            rstd = pool.tile([128, 1], F32, tag="lnrs")
            nc.scalar.activation(
                out=rstd[:], in_=mv[:, 1:2], func=Sqrt, bias=eps_t[:]
            )
            nc.vector.reciprocal(out=rstd[:], in_=rstd[:])
            nc.vector.tensor_scalar(
                out=dst[:], in0=src[:], scalar1=mv[:, 0:1], scalar2=rstd[:],
                op0=SUB, op1=MUL,
            )
            if g_b is not None:
                nc.vector.tensor_mul(out=dst[:], in0=dst[:], in1=g_b[:])
            if lb_b is not None:
                nc.vector.tensor_add(out=dst[:], in0=dst[:], in1=lb_b[:])

        # pool for phase-O/F tiles prefetched during P/A
        of = ctx.enter_context(tc.tile_pool(name="of", bufs=1))
        xr_s = of.tile([128, QT, D], F32)
        wo_t = of.tile([128, DT, D], BF16)
        h2_s = of.tile([128, QT, D], F32)
        h2T_s = of.tile([128, DT, QS], BF16)

        # -------- phases P+A share the big attention residents -------------
        with tc.tile_pool(name="pa", bufs=1) as pa, \
             tc.tile_pool(name="pad", bufs=1, space="DRAM") as pad:
            kT_s = pa.tile([128, 4, DT, QS], F8)        # K.T d-major, by rank
            vp_s = pa.tile([128, ST, H, HD + 1], F8)    # V natural + ones col
            qT_s = pa.tile([128, DT, QS], F8)           # Q.T (d-major)
            maskT_s = pa.tile([128, ST, QS], BF16)

            agk_in = pad.tile([1, 128 * DT * QS], F8, tag="agki")
            agk_out = pad.tile([4, 128 * DT * QS], F8, tag="agko")
            agv_in = pad.tile([1, 128 * 4 * H * (HD + 1)], F8, tag="agvi")
            agv_out = pad.tile([4, 128 * 4 * H * (HD + 1)], F8, tag="agvo")

            # ---- phase P: K/V/Q projections + AllGather -------------------
            with tc.tile_pool(name="pph", bufs=1) as pp, \
                 tc.tile_pool(name="pps", bufs=4, space="PSUM") as pps:
                xq_s = pp.tile([128, DT, QS], F8)
                wk_s = pp.tile([128, DT, D], F8, tag="w", name="wk_s")
                for k in range(DT):
                    nc.sync.dma_start(out=xq_s[:, k, :], in_=xqT8[:, k, :])
                nc.gpsimd.dma_start(
                    out=wk_s[:], in_=wk8.rearrange("(a p) n -> p a n", p=128)
                )
                bk_s = pp.tile([128, DT], F32)
                bq_s = pp.tile([128, DT], F32)
                nc.sync.dma_start(out=bk_s[:], in_=bk_c[:])
                nc.sync.dma_start(out=bq_s[:], in_=bq_c[:])
                wv_s = pp.tile([128, DT, D], F8, tag="w", name="wv_s")
                nc.gpsimd.dma_start(
                    out=wv_s[:], in_=wv8.rearrange("(a p) n -> p a n", p=128)
                )
                bv_b = bcast_load(pp, bv_r, D, "bvb")

                # K for own keys (DoubleRow fp8, K=256 per step)
                k_own = pp.tile([128, DT, QS], F8)
                for m in range(DT):
                    ps = pps.tile([128, QS], F32, tag="pj")
                    for kp in range(DT // 2):
                        nc.tensor.matmul(
                            ps[:],
                            wk_s[:, 2 * kp : 2 * kp + 2, m * 128 : (m + 1) * 128],
                            xq_s[:, 2 * kp : 2 * kp + 2, :],
                            start=(kp == 0), stop=(kp == DT // 2 - 1),
                            perf_mode=DR,
                        )
                    nc.vector.tensor_scalar(
                        out=k_own[:, m, :], in0=ps[:],
                        scalar1=bk_s[:, m : m + 1], scalar2=None, op0=ADD,
                    )
                nc.sync.dma_start(
                    out=agk_in[:].rearrange("o (p a q) -> (o p) a q", p=128, a=DT),
                    in_=k_own[:],
                )
                nc.gpsimd.collective_compute(
                    "AllGather", mybir.AluOpType.bypass,
                    ins=[agk_in[:]], outs=[agk_out[:]], replica_groups=GROUPS,
                )

                # V for own keys (natural layout + ones col), fp8
                v_own = pp.tile([128, 4, H, HD + 1], F8)
                nc.vector.memset(v_own[:, :, :, HD : HD + 1], 1.0)
                wq_s = pp.tile([128, DT, D], F8, tag="w", name="wq_s")
                nc.gpsimd.dma_start(
                    out=wq_s[:], in_=wq8.rearrange("(a p) n -> p a n", p=128)
                )
                for stl in range(4):
                    pss = [
                        pps.tile([128, 512], F32, tag="pj", name=f"vps{stl}_{c}")
                        for c in range(2)
                    ]
                    for kp in range(DT // 2):
                        for c in range(2):
                            nc.tensor.matmul(
                                pss[c][:],
                                xq_s[:, 2 * kp : 2 * kp + 2,
                                     stl * 128 : (stl + 1) * 128],
                                wv_s[:, 2 * kp : 2 * kp + 2,
                                     c * 512 : (c + 1) * 512],
                                start=(kp == 0), stop=(kp == DT // 2 - 1),
                                perf_mode=DR,
                            )
                    for c in range(2):
                        nc.vector.tensor_add(
                            out=v_own[:, stl, c * 8 : (c + 1) * 8, 0:HD],
                            in0=pss[c][:].rearrange("p (h e) -> p h e", e=HD),
                            in1=bv_b[:, c * 512 : (c + 1) * 512].rearrange(
                                "p (h e) -> p h e", e=HD
                            ),
                        )
                nc.sync.dma_start(
                    out=agv_in[:].rearrange(
                        "o (p s h e) -> (o p) s h e", p=128, s=4, h=H
                    ),
                    in_=v_own[:],
                )
                nc.gpsimd.collective_compute(
                    "AllGather", mybir.AluOpType.bypass,
                    ins=[agv_in[:]], outs=[agv_out[:]], replica_groups=GROUPS,
                )

                # Q (own rows)
                for m in range(DT):
                    ps = pps.tile([128, QS], F32, tag="pj")
                    for kp in range(DT // 2):
                        nc.tensor.matmul(
                            ps[:],
                            wq_s[:, 2 * kp : 2 * kp + 2, m * 128 : (m + 1) * 128],
                            xq_s[:, 2 * kp : 2 * kp + 2, :],
                            start=(kp == 0), stop=(kp == DT // 2 - 1),
                            perf_mode=DR,
                        )
                    nc.vector.tensor_scalar(
                        out=qT_s[:, m, :], in0=ps[:],
                        scalar1=bq_s[:, m : m + 1], scalar2=None, op0=ADD,
                    )

                # prefetches for later phases (independent of P/A compute)
                nc.sync.dma_start(
                    out=maskT_s[:], in_=maskT.rearrange("p a q -> p (a q)")
                    .rearrange("p (a q) -> p a q", a=ST),
                )
                nc.sync.dma_start(
                    out=xr_s[:], in_=xrows.rearrange("(t p) d -> p t d", p=128)
                )
                nc.gpsimd.dma_start(
                    out=wo_t[:], in_=wo.rearrange("(a p) n -> p a n", p=128)
                )

                # AllGather returns
                for r in range(4):
                    nc.sync.dma_start(
                        out=kT_s[:, r, :, :],
                        in_=agk_out[r : r + 1, :].rearrange(
                            "o (p a q) -> (o p) a q", p=128, a=DT
                        ),
                    )
                for r in range(4):
                    nc.sync.dma_start(
                        out=vp_s[:, 4 * r : 4 * r + 4, :, :],
                        in_=agv_out[r : r + 1, :].rearrange(
                            "o (p s h e) -> (o p) s h e", p=128, s=4, h=H
                        ),
                    )

            # ---- phase A: attention ---------------------------------------
            with tc.tile_pool(name="aph", bufs=1) as apl, \
                 tc.tile_pool(name="aqk", bufs=3, space="PSUM") as aqk, \
                 tc.tile_pool(name="avps", bufs=1, space="PSUM") as avps, \
                 tc.tile_pool(name="adr", bufs=1, space="DRAM") as adr:
                den_d = [
                    adr.tile([4, QS], F32, tag="dend", name=f"den_d{_b}", bufs=4)
                    for _b in range(4)
                ]
                den_d2 = [
                    adr.tile([4, QS], F32, tag="dend2", name=f"den_d2{_b}", bufs=4)
                    for _b in range(4)
                ]

                def normalize_batch(b, tail=False):
                    # reciprocal of this batch's 4 den rows ([32,64] shape for
                    # lane parallelism), then broadcast and scale 2 oT tiles
                    den_sb = apl.tile([32, 64], F32, tag="densb", name=f"densb{b}")
                    flat = den_d[b].rearrange("a q -> (a q)")
                    nc.sync.dma_start(
                        out=den_sb[:], in_=flat.rearrange("(p f) -> p f", f=64)
                    )
                    nc.vector.reciprocal(out=den_sb[:], in_=den_sb[:])
                    flat2 = den_d2[b].rearrange("a q -> (a q)")
                    nc.sync.dma_start(
                        out=flat2.rearrange("(p f) -> p f", f=64), in_=den_sb[:]
                    )
                    rb2 = apl.tile([128, 2, QS], F32, tag="rb2", name=f"rb2{b}")
                    dv = den_d2[b].rearrange("(a e) q -> e a q", e=2)
                    nc.sync.dma_start(
                        out=rb2[0:64, :, :],
                        in_=dv[0:1, :, :].broadcast_to((64, 2, QS)),
                    )
                    nc.sync.dma_start(
                        out=rb2[64:128, :, :],
                        in_=dv[1:2, :, :].broadcast_to((64, 2, QS)),
                    )
                    eng = nc.vector if tail else nc.gpsimd
                    for tt in range(2):
                        t = b * 2 + tt
                        eng.tensor_mul(
                            out=oT_s[:, t, :], in0=oT_s[:, t, :],
                            in1=rb2[:, tt, :],
                        )

                for m in range(DT):
                    avs = [
                        avps.tile([HD + 1, QS], F32, tag=f"av{_e}",
                                  name=f"av{m}_{_e}")
                        for _e in range(2)
                    ]
                    for jp in range(ST // 2):
                        pt = apl.tile([128, 2, 2, QS], F8, tag="pt", bufs=2)
                        for jj in range(2):
                            j = jp * 2 + jj
                            qk = aqk.tile([128, 2, 512], F32, tag="qk")
                            nc.tensor.matmul(
                                qk[:, 0, :],
                                kT_s[0:64, j // 4, m, (j % 4) * 128 : (j % 4) * 128 + 128],
                                qT_s[0:64, m, :],
                                start=True, stop=True,
                            )
                            nc.tensor.matmul(
                                qk[:, 1, :],
                                kT_s[64:128, j // 4, m, (j % 4) * 128 : (j % 4) * 128 + 128],
                                qT_s[64:128, m, :],
                                start=True, stop=True,
                            )
                            pe = apl.tile([128, 2, QS], BF16, tag="pe", bufs=3)
                            nc.scalar.activation(pe[:], qk[:], Exp, scale=0.125)
                            nc.vector.tensor_mul(
                                out=pt[:, jj, :, :], in0=pe[:],
                                in1=maskT_s[:, j, :][:, None, :].broadcast_to(
                                    (128, 2, QS)
                                ),
                            )
                        for e in range(2):
                            nc.tensor.matmul(
                                avs[e][:],
                                vp_s[:, 2 * jp : 2 * jp + 2, 2 * m + e, :],
                                pt[:, :, e, :],
                                start=(jp == 0), stop=(jp == ST // 2 - 1),
                                perf_mode=DR,
                            )
                    # stash denominator rows (via DRAM); evict unnormalized
                    for e in range(2):
                        h = 2 * m + e
                        dr_row = apl.tile([1, QS], F32, tag="dr", bufs=2)
                        nc.vector.tensor_copy(out=dr_row[:], in_=avs[e][HD : HD + 1, :])
                        nc.sync.dma_start(
                            out=den_d[h // 4][h % 4 : h % 4 + 1, :], in_=dr_row[:]
                        )
                        nc.vector.tensor_copy(
                            out=oT_s[e * 64 : e * 64 + 64, m, :], in_=avs[e][0:HD, :]
                        )
                    if m == 1:
                        normalize_batch(0)
                    elif m == 3:
                        normalize_batch(1)
                    elif m == 5:
                        normalize_batch(2)
                normalize_batch(3, tail=True)

        # -------- phase O: out-proj, AddNorm, LN2, transpose ---------------
        with tc.tile_pool(name="oph", bufs=1) as op, \
             tc.tile_pool(name="ops", bufs=4, space="PSUM") as ops, \
             tc.tile_pool(name="otps", bufs=2, space="PSUM") as otps, \
             tc.tile_pool(name="oln", bufs=4) as oln:
            bo_b = bcast_load(op, bo_r, D, "bob")
            g1_b = bcast_load(op, g1_r, D, "g1b")
            lb1_b = bcast_load(op, lb1r_r, D, "lb1b")   # ln1_b + r (host)

            h_s = op.tile([128, QT, D], F32)
            for mq in range(QT):
                nc.gpsimd.tensor_add(
                    out=xr_s[:, mq, :], in0=xr_s[:, mq, :], in1=bo_b[:]
                )
            for mq in range(QT):
                pss = [
                    ops.tile([128, 512], F32, tag="op", name=f"ops{mq}_{ns}")
                    for ns in range(D // 512)
                ]
                for k in range(DT):
                    for ns in range(D // 512):
                        nc.tensor.matmul(
                            pss[ns][:],
                            oT_s[:, k, mq * 128 : (mq + 1) * 128],
                            wo_t[:, k, ns * 512 : (ns + 1) * 512],
                            start=(k == 0), stop=(k == DT - 1),
                        )
                for ns in range(D // 512):
                    sl = slice(ns * 512, (ns + 1) * 512)
                    nc.vector.tensor_add(
                        out=h_s[:, mq, sl], in0=pss[ns][:], in1=xr_s[:, mq, sl]
                    )
                layer_norm(oln, h_s[:, mq, :], h_s[:, mq, :], g1_b, lb1_b)
                # h2n = pure-normalized LN2; gamma2/beta2 folded into w1/b1
                # (host) and into the FF2-tail residual
                layer_norm(oln, h2_s[:, mq, :], h_s[:, mq, :])
                h2bf = oln.tile([128, D], BF16, tag="h2bf")
                nc.scalar.copy(out=h2bf[:], in_=h2_s[:, mq, :])
                for t in range(DT):
                    tp = otps.tile([128, 128], BF16, tag="tp")
                    nc.tensor.transpose(
                        tp[:], h2bf[:, t * 128 : (t + 1) * 128], ident[:]
                    )
                    nc.scalar.copy(
                        out=h2T_s[:, t, mq * 128 : (mq + 1) * 128], in_=tp[:]
                    )

        # -------- phase F: FFN + AddNorm -----------------------------------
        with tc.tile_pool(name="fph", bufs=1) as fp, \
             tc.tile_pool(name="fln", bufs=4) as fln:
            b1_s = fp.tile([128, FT], F32)
            nc.sync.dma_start(out=b1_s[:], in_=b1_c[:])
            g2_b = bcast_load(fp, g2_r, D, "g2b")
            cb_b = bcast_load(fp, cb_r, D, "cbb")   # lb2 + b2 (host)
            g3_b = bcast_load(fp, g3_r, D, "g3b")
            lb3_b = bcast_load(fp, lb3_r, D, "lb3b")
            # h2full = h2n * g2 + (lb2 + b2), off the critical path
            for mq in range(QT):
                nc.gpsimd.tensor_mul(
                    out=h2_s[:, mq, :], in0=h2_s[:, mq, :], in1=g2_b[:]
                )
                nc.gpsimd.tensor_add(
                    out=h2_s[:, mq, :], in0=h2_s[:, mq, :], in1=cb_b[:]
                )

            fT_s = fp.tile([128, FT, QS], BF16)
            # FF1: fT[:, mf, :] = relu(w1[:, mf].T @ h2T + b1)
            with tc.tile_pool(name="fw1", bufs=2) as fw1, \
                 tc.tile_pool(name="fps", bufs=3, space="PSUM") as fps:
                for mfg in range(4):
                    w1_t = fw1.tile([128, DT, 1024], BF16, tag="w1")
                    nc.gpsimd.dma_start(
                        out=w1_t[:],
                        in_=w1.rearrange("(a p) n -> p a n", p=128)[
                            :, :, mfg * 1024 : (mfg + 1) * 1024
                        ],
                    )
                    for mfl in range(8):
                        mf = mfg * 8 + mfl
                        ps = fps.tile([128, QS], F32, tag="f1")
                        for k in range(DT):
                            nc.tensor.matmul(
                                ps[:],
                                w1_t[:, k, mfl * 128 : (mfl + 1) * 128],
                                h2T_s[:, k, :],
                                start=(k == 0), stop=(k == DT - 1),
                            )
                        nc.scalar.activation(
                            out=fT_s[:, mf, :], in_=ps[:], func=Relu,
                            bias=b1_s[:, mf : mf + 1],
                        )

            # FF2 in two mq-halves so the first half's LN3/output
            # overlaps the second half's matmuls
            out_t = fp.tile([128, QT, D], F32)
            with tc.tile_pool(name="fw2", bufs=3) as fw2, \
                 tc.tile_pool(name="f2ps", bufs=8, space="PSUM") as f2ps:
                for half in range(2):
                    mqs = (0, 1, 2) if half == 0 else (3,)
                    ps2 = [
                        f2ps.tile([128, 512], F32, tag="f2",
                                  name=f"ps2_{half}_{_i}")
                        for _i in range(2 * len(mqs))
                    ]
                    for kf in range(FT):
                        w2_t = fw2.tile([128, D], BF16, tag="w2",
                                        name=f"w2_t{half}_{kf}")
                        nc.gpsimd.dma_start(
                            out=w2_t[:], in_=w2[kf * 128 : (kf + 1) * 128, :]
                        )
                        for i, mq in enumerate(mqs):
                            for ns in range(D // 512):
                                nc.tensor.matmul(
                                    ps2[i * 2 + ns][:],
                                    fT_s[:, kf, mq * 128 : (mq + 1) * 128],
                                    w2_t[:, ns * 512 : (ns + 1) * 512],
                                    start=(kf == 0), stop=(kf == FT - 1),
                                )
                    for i, mq in enumerate(mqs):
                        for ns in range(D // 512):
                            sl = slice(ns * 512, (ns + 1) * 512)
                            nc.vector.tensor_add(
                                out=out_t[:, mq, sl], in0=ps2[i * 2 + ns][:],
                                in1=h2_s[:, mq, sl],
                            )
                        layer_norm(
                            fln, out_t[:, mq, :], out_t[:, mq, :], g3_b, lb3_b
                        )
                        nc.sync.dma_start(
                            out=out_d.rearrange("(t p) d -> p t d", p=128)[:, mq, :],
                            in_=out_t[:, mq, :],
                        )

    _split_sync_waits(nc)
    return nc


_NC_CACHE = None


def _get_program():
    global _NC_CACHE
    if _NC_CACHE is None:
        _NC_CACHE = _build_program()
    return _NC_CACHE


# ----------------------------------------------------------------------------
# host wrapper
# ----------------------------------------------------------------------------

def _col_interleave(v, nt):
    """[n] f32 -> [128, nt] where col j holds v[j*128:(j+1)*128]."""
    return np.ascontiguousarray(
        np.asarray(v, np.float32).reshape(nt, 128).T
    )


def kernel(**inputs):
    x = np.asarray(inputs["cur_input"], np.float32)          # [B, S, D]
    cls = np.asarray(inputs["classVector"], np.float32)      # [B, 1, 10]
    mask = np.asarray(inputs["attn_mask"])                   # [S, S] bool

    bf = lambda a: np.ascontiguousarray(np.asarray(a, np.float32)).astype(NP_BF16)
    f8 = lambda a: np.ascontiguousarray(np.asarray(a, np.float32)).astype(NP_F8)
    f32 = lambda a: np.ascontiguousarray(np.asarray(a, np.float32))
    row = lambda v: f32(np.asarray(v, np.float32).reshape(1, -1))

    # cross-attn over a single key collapses to a query-independent row:
    # r = ((cls @ ce_w + ce_b) @ ca_wv + ca_bv) @ ca_wo + ca_bo  (per batch)
    cv = cls[:, 0, :] @ np.asarray(inputs["ce_w"], np.float32) + np.asarray(
        inputs["ce_b"], np.float32
    )
    vcv = cv @ np.asarray(inputs["ca_wv"], np.float32) + np.asarray(
        inputs["ca_bv"], np.float32
    )
    r_rows = vcv @ np.asarray(inputs["ca_wo"], np.float32) + np.asarray(
        inputs["ca_bo"], np.float32
    )  # [B, D]
    lb1 = np.asarray(inputs["ln1_b"], np.float32)

    shared = dict(
        wq8=f8(inputs["sa_wq"]),
        wk8=f8(inputs["sa_wk"]),
        wv8=f8(inputs["sa_wv"]),
        wo=bf(inputs["sa_wo"]),
        # gamma2 folded into w1 rows; lb2 folded into b1 (and into cb_r below)
        w1=bf(np.asarray(inputs["ff_w1"], np.float32)
              * np.asarray(inputs["ln2_g"], np.float32)[:, None]),
        w2=bf(inputs["ff_w2"]),
        bq_c=_col_interleave(inputs["sa_bq"], DT),
        bk_c=_col_interleave(inputs["sa_bk"], DT),
        b1_c=_col_interleave(
            np.asarray(inputs["ff_b1"], np.float32)
            + np.asarray(inputs["ln2_b"], np.float32)
            @ np.asarray(inputs["ff_w1"], np.float32), FT),
        bv_r=row(inputs["sa_bv"]),
        bo_r=row(inputs["sa_bo"]),
        cb_r=row(np.asarray(inputs["ff_b2"], np.float32)
                 + np.asarray(inputs["ln2_b"], np.float32)),
        g1_r=row(inputs["ln1_g"]),
        g2_r=row(inputs["ln2_g"]),
        g3_r=row(inputs["ln3_g"]),
        lb3_r=row(inputs["ln3_b"]),
    )

    mT = mask.T.astype(np.float32)  # [S key, S query]
    in_maps = []
    for c in range(NCORES):
        b, q0 = c // (NCORES // B), (c % (NCORES // B)) * QS
        xTb = x[b].T.reshape(DT, 128, S).transpose(1, 0, 2)       # [128, DT, S]
        mTc = mT[:, q0 : q0 + QS].reshape(ST, 128, QS).transpose(1, 0, 2)
        in_maps.append(
            dict(
                shared,
                xqT8=f8(xTb[:, :, q0 : q0 + QS]),
                xrows=f32(x[b, q0 : q0 + QS, :]),
                maskT=bf(mTc),
                lb1r_r=row(lb1 + r_rows[b]),
            )
        )

    res = run_bass_kernel_spmd(_get_program(), in_maps, list(range(NCORES)))
    out = np.empty((B, S, D), np.float32)
    for c in range(NCORES):
        b, q0 = c // (NCORES // B), (c % (NCORES // B)) * QS
        out[b, q0 : q0 + QS] = res.results[c]["out"]
    return out


# revision 16
# speedup vs baseline: 1.2189x; 1.0382x over previous
"""Trainium2 Bass kernel for nn_Block_86672440033530 (sparse_attention).

Transformer block: masked self-attention + AddNorm, class-vector cross-attn
(collapses to a host-computed broadcast row since Sk=1) + AddNorm, FFN + AddNorm.

Sharding: 8 cores = 2 batches x 4 query-blocks of 512 rows. K/V projections are
sharded across each 4-core batch group (each core projects its own 512 keys)
and exchanged with two AllGather collectives; everything else is row-local.

Precision: Q/K/V projections, QK, and AV run in fp8(e4m3) — DoubleRow fp8
matmuls for the projections and AV (K=256 per step), row-tiled K=64 matmul
pairs for QK (two heads concurrently in the PE array). The attention output is
a tiny fraction of the residual stream here, so fp8 error washes out. O-proj
and the FFN stay bf16 (they carry ~half the stream); PSUM always fp32. The
softmax denominator comes from a ones-column appended to V (fused into the AV
matmul) and is divided out via a DRAM-bounce partition broadcast.
"""
import contextlib
import ctypes
import sys
import types

import numpy as np

if "/opt/trn_rl_repo" not in sys.path:
    sys.path.insert(0, "/opt/trn_rl_repo")

import ml_dtypes  # noqa: E402
import concourse.bass as bass  # noqa: E402
import concourse.mybir as mybir  # noqa: E402
import concourse.tile as tile  # noqa: E402
from concourse.bass_utils import run_bass_kernel_spmd  # noqa: E402
from concourse.masks import make_identity  # noqa: E402

BF16 = mybir.dt.bfloat16
F32 = mybir.dt.float32
F8 = mybir.dt.float8e4
NP_BF16 = ml_dtypes.bfloat16
NP_F8 = ml_dtypes.float8_e4m3

B, S, D, H, DFF = 2, 2048, 1024, 16, 4096
HD = D // H                      # 64
NCORES = 8
QS = S // (NCORES // B)          # 512 query rows per core
QT = QS // 128                   # 4 query tiles per core
DT = D // 128                    # 8 d-blocks
ST = S // 128                    # 16 key tiles
FT = DFF // 128                  # 32 dff tiles
EPS = 1e-5
GROUPS = [[0, 1, 2, 3], [4, 5, 6, 7]]
DR = mybir.MatmulPerfMode.DoubleRow


def _install_ntff_shim():
    """The axon image lacks antenv.axon_hooks; register the NTFF profile hook
    via ctypes so run_bass_kernel_spmd(trace=True) works. Harmless if unused."""
    try:
        import antenv
    except ImportError:
        return
    if "antenv.axon_hooks" in sys.modules:
        return

    def _make_hook(so_path):
        try:
            lib = ctypes.CDLL(so_path)
        except OSError:
            return None
        if not hasattr(lib, "axon_start_nrt_profile"):
            return None
        lib.axon_start_nrt_profile.argtypes = [
            ctypes.POINTER(ctypes.c_int64),
            ctypes.c_size_t,
        ]
        lib.axon_start_nrt_profile.restype = ctypes.c_int64
        lib.axon_stop_nrt_profile.argtypes = [ctypes.c_char_p]
        lib.axon_stop_nrt_profile.restype = ctypes.c_int64

        @contextlib.contextmanager
        def _hook(output_dir, device_ids):
            import jax

            jax.devices()
            if device_ids:
                ids = (ctypes.c_int64 * len(device_ids))(*device_ids)
                rc = lib.axon_start_nrt_profile(ids, len(device_ids))
            else:
                rc = lib.axon_start_nrt_profile(None, 0)
            if rc != 0:
                raise RuntimeError(f"axon_start_nrt_profile rc={rc}")
            try:
                yield
            finally:
                n = lib.axon_stop_nrt_profile(str(output_dir).encode())
                print(f"profile: {n} file(s) -> {output_dir}", file=sys.stderr)

        return _hook

    m = types.ModuleType("antenv.axon_hooks")
    m._hook = _make_hook("/opt/axon/libaxon_pjrt.so")
    m.set_axon_ntff_profile_hook = lambda h: setattr(m, "_hook", h)
    m.get_axon_ntff_profile_hook = lambda: m._hook
    sys.modules["antenv.axon_hooks"] = m
    import antenv

    antenv.axon_hooks = m


_install_ntff_shim()


def _split_sync_waits(nc, limit=1):
    """This walrus build accepts at most one sync-wait command per
    instruction; move excess waits onto same-engine NoOps placed before."""
    for func in nc.m.functions:
        for bb in func.blocks:
            out = []
            for ins in bb.instructions:
                si = getattr(ins, "sync_info", None)
                waits = list(si.on_wait) if (si is not None and si.on_wait) else []
                if len(waits) > limit:
                    keep, move = waits[:limit], waits[limit:]
                    for i in range(0, len(move), limit):
                        out.append(
                            mybir.InstNoOp(
                                name=f"{ins.name}-wsplit{i}",
                                sync_info=mybir.SyncInfo(
                                    on_wait=move[i : i + limit], on_update=[]
                                ),
                                bass_nofuse=True,
                                engine=ins.engine,
                            )
                        )
                    si.on_wait = keep
                out.append(ins)
            bb.instructions[:] = out


# ----------------------------------------------------------------------------
# device program (SPMD; identical on all 8 cores, per-core data differs)
# ----------------------------------------------------------------------------

def _build_program():
    nc = bass.Bass()

    def din(name, shape, dt):
        return nc.dram_tensor(name, list(shape), dt, kind="ExternalInput")

    # per-core tensors
    xT8 = din("xT8", [128, DT, S], F8)          # x[b].T full (d-major), fp8
    xqT8 = din("xqT8", [128, DT, QS], F8)       # own rows of x[b].T, fp8
    xrows = din("xrows", [QS, D], F32)          # own rows + bo (residual, host)
    maskT = din("maskT", [128, ST, QS], BF16)   # mask.T own q cols {0,1}
    # weights
    wq8 = din("wq8", [D, D], F8)
    wk8 = din("wk8", [D, D], F8)
    wv8 = din("wv8", [D, D], F8)
    wo = din("wo", [D, D], BF16)
    w1 = din("w1", [D, DFF], BF16)              # gamma2 folded into rows
    w2 = din("w2", [DFF, D], BF16)
    # f32 bias/ln vectors: column-interleaved [128, n] or rows [1, n]
    bq_c = din("bq_c", [128, DT], F32)
    bk_c = din("bk_c", [128, DT], F32)
    b1_c = din("b1_c", [128, FT], F32)
    bv_r = din("bv_r", [1, D], F32)
    cb_r = din("cb_r", [1, D], F32)             # lb2 + b2 (host)
    g1_r = din("g1_r", [1, D], F32)
    lb1r_r = din("lb1r_r", [1, D], F32)         # ln1_b + cross-attn row (host)
    g2_r = din("g2_r", [1, D], F32)
    g3_r = din("g3_r", [1, D], F32)
    lb3_r = din("lb3_r", [1, D], F32)

    out_d = nc.dram_tensor("out", [QS, D], F32, kind="ExternalOutput")

    Exp = mybir.ActivationFunctionType.Exp
    Relu = mybir.ActivationFunctionType.Relu
    Sqrt = mybir.ActivationFunctionType.Sqrt
    ADD = mybir.AluOpType.add
    SUB = mybir.AluOpType.subtract
    MUL = mybir.AluOpType.mult

    with tile.TileContext(nc) as tc, contextlib.ExitStack() as ctx:
        # -------- whole-kernel residents (small) ---------------------------
        res = ctx.enter_context(tc.tile_pool(name="res", bufs=1))

        ident = res.tile([128, 128], BF16)
        make_identity(nc, ident)
        eps_t = res.tile([128, 1], F32)
        nc.vector.memset(eps_t[:], EPS)
        oT_s = res.tile([128, DT, QS], BF16)     # attention output (transposed)

        def bcast_load(pool, src_row, n, tag, eng=None):
            t = pool.tile([128, n], F32, tag=tag)
            e = eng if eng is not None else nc.sync
            e.dma_start(out=t[:], in_=src_row[0:1, :].broadcast_to((128, n)))
            return t

        def layer_norm(pool, dst, src, g_b=None, lb_b=None):
            """dst = LN_freedim(src) [* g] [+ b] for [128, D] f32 views."""
            stats = pool.tile([128, 2, 6], F32, tag="lnst")
            mv = pool.tile([128, 2], F32, tag="lnmv")
            for sg in range(2):
                nc.vector.bn_stats(
                    out=stats[:, sg, :], in_=src[:, sg * 512 : (sg + 1) * 512]
                )
            nc.vector.bn_aggr(out=mv[:], in_=stats[:])
            rstd = pool.tile([128, 1], F32, tag="lnrs")
            nc.scalar.activation(
                out=rstd[:], in_=mv[:, 1:2], func=Sqrt, bias=eps_t[:]
            )
            nc.vector.reciprocal(out=rstd[:], in_=rstd[:])
            nc.vector.tensor_scalar(
                out=dst[:], in0=src[:], scalar1=mv[:, 0:1], scalar2=rstd[:],
                op0=SUB, op1=MUL,
            )
            if g_b is not None:
                nc.vector.tensor_mul(out=dst[:], in0=dst[:], in1=g_b[:])
            if lb_b is not None:
                nc.vector.tensor_add(out=dst[:], in0=dst[:], in1=lb_b[:])

        # pool for phase-O/F tiles prefetched during P/A
        of = ctx.enter_context(tc.tile_pool(name="of", bufs=1))
        xr_s = of.tile([128, QT, D], F32)
        wo_t = of.tile([128, DT, D], BF16)
        h2_s = of.tile([128, QT, D], F32)
        h2T_s = of.tile([128, DT, QS], BF16)
        fw1 = ctx.enter_context(tc.tile_pool(name="fw1", bufs=2))
        w1_pre = fw1.tile([128, DT, 1024], BF16, tag="w1", name="w1_pre")

        # -------- phases P+A share the big attention residents -------------
        with tc.tile_pool(name="pa", bufs=1) as pa, \
             tc.tile_pool(name="pad", bufs=1, space="DRAM") as pad:
            kT_s = pa.tile([128, 4, DT, QS], F8)        # K.T d-major, by rank
            vp_s = pa.tile([128, ST, H, HD + 1], F8)    # V natural + ones col
            qT_s = pa.tile([128, DT, QS], F8)           # Q.T (d-major)
            maskT_s = pa.tile([128, ST, QS], BF16)

            agk_in = pad.tile([1, 128 * DT * QS], F8, tag="agki")
            agk_out = pad.tile([4, 128 * DT * QS], F8, tag="agko")

            # ---- phase P: K own-shard + AllGather; V/Q local --------------
            with tc.tile_pool(name="pph", bufs=1) as pp, \
                 tc.tile_pool(name="pps", bufs=4, space="PSUM") as pps:
                x_s = pp.tile([128, DT, S], F8)
                xq_s = pp.tile([128, DT, QS], F8)
                wk_s = pp.tile([128, DT, D], F8, tag="w", name="wk_s")
                nc.gpsimd.dma_start(
                    out=wk_s[:], in_=wk8.rearrange("(a p) n -> p a n", p=128)
                )
                for k in range(DT):
                    nc.sync.dma_start(out=xq_s[:, k, :], in_=xqT8[:, k, :])
                bk_s = pp.tile([128, DT], F32)
                bq_s = pp.tile([128, DT], F32)
                nc.sync.dma_start(out=bk_s[:], in_=bk_c[:])
                nc.sync.dma_start(out=bq_s[:], in_=bq_c[:])
                wv_s = pp.tile([128, DT, D], F8, tag="w", name="wv_s")
                nc.gpsimd.dma_start(
                    out=wv_s[:], in_=wv8.rearrange("(a p) n -> p a n", p=128)
                )
                for k in range(DT):
                    nc.sync.dma_start(out=x_s[:, k, :], in_=xT8[:, k, :])
                bv_b = bcast_load(pp, bv_r, D, "bvb")

                # K for own keys only (DoubleRow fp8, K=256 per step)
                k_own = pp.tile([128, DT, QS], F8)
                xq = xq_s[:]
                for m in range(DT):
                    ps = pps.tile([128, QS], F32, tag="pj")
                    for kp in range(DT // 2):
                        nc.tensor.matmul(
                            ps[:],
                            wk_s[:, 2 * kp : 2 * kp + 2, m * 128 : (m + 1) * 128],
                            xq[:, 2 * kp : 2 * kp + 2, :],
                            start=(kp == 0), stop=(kp == DT // 2 - 1),
                            perf_mode=DR,
                        )
                    nc.vector.tensor_scalar(
                        out=k_own[:, m, :], in0=ps[:],
                        scalar1=bk_s[:, m : m + 1], scalar2=None, op0=ADD,
                    )
                nc.sync.dma_start(
                    out=agk_in[:].rearrange("o (p a q) -> (o p) a q", p=128, a=DT),
                    in_=k_own[:],
                )
                nc.gpsimd.collective_compute(
                    "AllGather", mybir.AluOpType.bypass,
                    ins=[agk_in[:]], outs=[agk_out[:]], replica_groups=GROUPS,
                )

                # V for ALL keys (natural layout + ones col), fp8, local
                nc.vector.memset(vp_s[:, :, :, HD : HD + 1], 1.0)
                wq_s = pp.tile([128, DT, D], F8, tag="w", name="wq_s")
                nc.gpsimd.dma_start(
                    out=wq_s[:], in_=wq8.rearrange("(a p) n -> p a n", p=128)
                )
                for st in range(ST):
                    pss = [
                        pps.tile([128, 512], F32, tag="pj", name=f"vps{st}_{c}")
                        for c in range(2)
                    ]
                    for kp in range(DT // 2):
                        for c in range(2):
                            nc.tensor.matmul(
                                pss[c][:],
                                x_s[:, 2 * kp : 2 * kp + 2,
                                    st * 128 : (st + 1) * 128],
                                wv_s[:, 2 * kp : 2 * kp + 2,
                                     c * 512 : (c + 1) * 512],
                                start=(kp == 0), stop=(kp == DT // 2 - 1),
                                perf_mode=DR,
                            )
                    for c in range(2):
                        nc.vector.tensor_add(
                            out=vp_s[:, st, c * 8 : (c + 1) * 8, 0:HD],
                            in0=pss[c][:].rearrange("p (h e) -> p h e", e=HD),
                            in1=bv_b[:, c * 512 : (c + 1) * 512].rearrange(
                                "p (h e) -> p h e", e=HD
                            ),
                        )
                    if st == 1:
                        # prefetches for later phases (off the DMA rush hour)
                        nc.sync.dma_start(
                            out=maskT_s[:],
                            in_=maskT.rearrange("p a q -> p (a q)")
                            .rearrange("p (a q) -> p a q", a=ST),
                        )
                    if st == 3:
                        nc.sync.dma_start(
                            out=xr_s[:],
                            in_=xrows.rearrange("(t p) d -> p t d", p=128),
                        )
                    if st == 5:
                        nc.sync.dma_start(
                            out=wo_t[:],
                            in_=wo.rearrange("(a p) n -> p a n", p=128),
                        )

                # Q (own rows)
                for m in range(DT):
                    ps = pps.tile([128, QS], F32, tag="pj")
                    for kp in range(DT // 2):
                        nc.tensor.matmul(
                            ps[:],
                            wq_s[:, 2 * kp : 2 * kp + 2, m * 128 : (m + 1) * 128],
                            xq[:, 2 * kp : 2 * kp + 2, :],
                            start=(kp == 0), stop=(kp == DT // 2 - 1),
                            perf_mode=DR,
                        )
                    nc.vector.tensor_scalar(
                        out=qT_s[:, m, :], in0=ps[:],
                        scalar1=bq_s[:, m : m + 1], scalar2=None, op0=ADD,
                    )

                # AllGather return
                for r in range(4):
                    nc.sync.dma_start(
                        out=kT_s[:, r, :, :],
                        in_=agk_out[r : r + 1, :].rearrange(
                            "o (p a q) -> (o p) a q", p=128, a=DT
                        ),
                    )

            # w1 group-0 prefetch rides the idle DMA lanes during attention
            nc.sync.dma_start(
                out=w1_pre[:],
                in_=w1.rearrange("(a p) n -> p a n", p=128)[:, :, 0:1024],
            )

            # ---- phase A: attention ---------------------------------------
            with tc.tile_pool(name="aph", bufs=1) as apl, \
                 tc.tile_pool(name="aqk", bufs=3, space="PSUM") as aqk, \
                 tc.tile_pool(name="avps", bufs=1, space="PSUM") as avps, \
                 tc.tile_pool(name="adr", bufs=1, space="DRAM") as adr:
                den_d = [
                    adr.tile([4, QS], F32, tag="dend", name=f"den_d{_b}", bufs=4)
                    for _b in range(4)
                ]
                den_d2 = [
                    adr.tile([4, QS], F32, tag="dend2", name=f"den_d2{_b}", bufs=4)
                    for _b in range(4)
                ]

                def normalize_batch(b, tail=False):
                    # reciprocal of this batch's 4 den rows ([32,64] shape for
                    # lane parallelism), then broadcast and scale 2 oT tiles
                    den_sb = apl.tile([32, 64], F32, tag="densb", name=f"densb{b}")
                    flat = den_d[b].rearrange("a q -> (a q)")
                    nc.sync.dma_start(
                        out=den_sb[:], in_=flat.rearrange("(p f) -> p f", f=64)
                    )
                    nc.vector.reciprocal(out=den_sb[:], in_=den_sb[:])
                    flat2 = den_d2[b].rearrange("a q -> (a q)")
                    nc.sync.dma_start(
                        out=flat2.rearrange("(p f) -> p f", f=64), in_=den_sb[:]
                    )
                    rb2 = apl.tile([128, 2, QS], F32, tag="rb2", name=f"rb2{b}")
                    dv = den_d2[b].rearrange("(a e) q -> e a q", e=2)
                    nc.sync.dma_start(
                        out=rb2[0:64, :, :],
                        in_=dv[0:1, :, :].broadcast_to((64, 2, QS)),
                    )
                    nc.sync.dma_start(
                        out=rb2[64:128, :, :],
                        in_=dv[1:2, :, :].broadcast_to((64, 2, QS)),
                    )
                    eng = nc.vector if tail else nc.gpsimd
                    for tt in range(2):
                        t = b * 2 + tt
                        eng.tensor_mul(
                            out=oT_s[:, t, :], in0=oT_s[:, t, :],
                            in1=rb2[:, tt, :],
                        )

                for m in range(DT):
                    avs = [
                        avps.tile([HD + 1, QS], F32, tag=f"av{_e}",
                                  name=f"av{m}_{_e}")
                        for _e in range(2)
                    ]
                    for jp in range(ST // 2):
                        pt = apl.tile([128, 2, 2, QS], F8, tag="pt", bufs=2)
                        pe = apl.tile([128, 2, 2, QS], BF16, tag="pe", bufs=2)
                        for jj in range(2):
                            j = jp * 2 + jj
                            qk = aqk.tile([128, 2, 512], F32, tag="qk")
                            nc.tensor.matmul(
                                qk[:, 0, :],
                                kT_s[0:64, j // 4, m, (j % 4) * 128 : (j % 4) * 128 + 128],
                                qT_s[0:64, m, :],
                                start=True, stop=True,
                            )
                            nc.tensor.matmul(
                                qk[:, 1, :],
                                kT_s[64:128, j // 4, m, (j % 4) * 128 : (j % 4) * 128 + 128],
                                qT_s[64:128, m, :],
                                start=True, stop=True,
                            )
                            nc.scalar.activation(
                                pe[:, jj, :, :], qk[:], Exp, scale=0.125
                            )
                        nc.vector.tensor_mul(
                            out=pt[:], in0=pe[:],
                            in1=maskT_s[:, 2 * jp : 2 * jp + 2, :][
                                :, :, None, :
                            ].broadcast_to((128, 2, 2, QS)),
                        )
                        for e in range(2):
                            nc.tensor.matmul(
                                avs[e][:],
                                vp_s[:, 2 * jp : 2 * jp + 2, 2 * m + e, :],
                                pt[:, :, e, :],
                                start=(jp == 0), stop=(jp == ST // 2 - 1),
                                perf_mode=DR,
                            )
                    # stash denominator rows (via DRAM); evict unnormalized
                    for e in range(2):
                        h = 2 * m + e
                        dr_row = apl.tile([1, QS], F32, tag="dr", bufs=2)
                        nc.vector.tensor_copy(out=dr_row[:], in_=avs[e][HD : HD + 1, :])
                        nc.sync.dma_start(
                            out=den_d[h // 4][h % 4 : h % 4 + 1, :], in_=dr_row[:]
                        )
                        nc.vector.tensor_copy(
                            out=oT_s[e * 64 : e * 64 + 64, m, :], in_=avs[e][0:HD, :]
                        )
                    if m == 1:
                        normalize_batch(0)
                    elif m == 3:
                        normalize_batch(1)
                    elif m == 5:
                        normalize_batch(2)
                normalize_batch(3, tail=True)

        # -------- phase O: out-proj, AddNorm, LN2, transpose ---------------
        with tc.tile_pool(name="oph", bufs=1) as op, \
             tc.tile_pool(name="ops", bufs=4, space="PSUM") as ops, \
             tc.tile_pool(name="otps", bufs=2, space="PSUM") as otps, \
             tc.tile_pool(name="oln", bufs=4) as oln:
            g1_b = bcast_load(op, g1_r, D, "g1b")
            lb1_b = bcast_load(op, lb1r_r, D, "lb1b")   # ln1_b + r (host)

            h_s = op.tile([128, QT, D], F32)
            for mq in range(QT):
                pss = [
                    ops.tile([128, 512], F32, tag="op", name=f"ops{mq}_{ns}")
                    for ns in range(D // 512)
                ]
                for k in range(DT):
                    for ns in range(D // 512):
                        nc.tensor.matmul(
                            pss[ns][:],
                            oT_s[:, k, mq * 128 : (mq + 1) * 128],
                            wo_t[:, k, ns * 512 : (ns + 1) * 512],
                            start=(k == 0), stop=(k == DT - 1),
                        )
                for ns in range(D // 512):
                    sl = slice(ns * 512, (ns + 1) * 512)
                    nc.vector.tensor_add(
                        out=h_s[:, mq, sl], in0=pss[ns][:], in1=xr_s[:, mq, sl]
                    )
                layer_norm(oln, h_s[:, mq, :], h_s[:, mq, :], g1_b, lb1_b)
                # h2n = pure-normalized LN2; gamma2/beta2 folded into w1/b1
                # (host) and into the FF2-tail residual
                layer_norm(oln, h2_s[:, mq, :], h_s[:, mq, :])
                h2bf = oln.tile([128, D], BF16, tag="h2bf")
                nc.scalar.copy(out=h2bf[:], in_=h2_s[:, mq, :])
                for t in range(DT):
                    tp = otps.tile([128, 128], BF16, tag="tp")
                    nc.tensor.transpose(
                        tp[:], h2bf[:, t * 128 : (t + 1) * 128], ident[:]
                    )
                    nc.scalar.copy(
                        out=h2T_s[:, t, mq * 128 : (mq + 1) * 128], in_=tp[:]
                    )

        # -------- phase F: FFN + AddNorm -----------------------------------
        with tc.tile_pool(name="fph", bufs=1) as fp, \
             tc.tile_pool(name="fln", bufs=4) as fln:
            b1_s = fp.tile([128, FT], F32)
            nc.sync.dma_start(out=b1_s[:], in_=b1_c[:])
            g2_b = bcast_load(fp, g2_r, D, "g2b")
            cb_b = bcast_load(fp, cb_r, D, "cbb")   # lb2 + b2 (host)
            g3_b = bcast_load(fp, g3_r, D, "g3b")
            lb3_b = bcast_load(fp, lb3_r, D, "lb3b")
            # h2full = h2n * g2 + (lb2 + b2), off the critical path
            for mq in range(QT):
                nc.gpsimd.tensor_mul(
                    out=h2_s[:, mq, :], in0=h2_s[:, mq, :], in1=g2_b[:]
                )
                nc.gpsimd.tensor_add(
                    out=h2_s[:, mq, :], in0=h2_s[:, mq, :], in1=cb_b[:]
                )

            fT_s = fp.tile([128, FT, QS], BF16)
            # FF1: fT[:, mf, :] = relu(w1[:, mf].T @ h2T + b1)
            with tc.tile_pool(name="fps", bufs=3, space="PSUM") as fps:
                for mfg in range(4):
                    if mfg == 0:
                        w1_t = w1_pre
                    else:
                        w1_t = fw1.tile(
                            [128, DT, 1024], BF16, tag="w1", name=f"w1_t{mfg}"
                        )
                        nc.sync.dma_start(
                            out=w1_t[:],
                            in_=w1.rearrange("(a p) n -> p a n", p=128)[
                                :, :, mfg * 1024 : (mfg + 1) * 1024
                            ],
                        )
                    for mfl in range(8):
                        mf = mfg * 8 + mfl
                        ps = fps.tile([128, QS], F32, tag="f1")
                        for k in range(DT):
                            nc.tensor.matmul(
                                ps[:],
                                w1_t[:, k, mfl * 128 : (mfl + 1) * 128],
                                h2T_s[:, k, :],
                                start=(k == 0), stop=(k == DT - 1),
                            )
                        nc.scalar.activation(
                            out=fT_s[:, mf, :], in_=ps[:], func=Relu,
                            bias=b1_s[:, mf : mf + 1],
                        )

            # FF2 in two mq-halves so the first half's LN3/output
            # overlaps the second half's matmuls
            out_t = fp.tile([128, QT, D], F32)
            with tc.tile_pool(name="fw2", bufs=3) as fw2, \
                 tc.tile_pool(name="f2ps", bufs=8, space="PSUM") as f2ps:
                for half in range(2):
                    mqs = (0, 1) if half == 0 else (2, 3)
                    ps2 = [
                        f2ps.tile([128, 512], F32, tag="f2",
                                  name=f"ps2_{half}_{_i}")
                        for _i in range(2 * len(mqs))
                    ]
                    for kf in range(FT):
                        w2_t = fw2.tile([128, D], BF16, tag="w2",
                                        name=f"w2_t{half}_{kf}")
                        nc.sync.dma_start(
                            out=w2_t[:], in_=w2[kf * 128 : (kf + 1) * 128, :]
                        )
                        for i, mq in enumerate(mqs):
                            for ns in range(D // 512):
                                nc.tensor.matmul(
                                    ps2[i * 2 + ns][:],
                                    fT_s[:, kf, mq * 128 : (mq + 1) * 128],
                                    w2_t[:, ns * 512 : (ns + 1) * 512],
                                    start=(kf == 0), stop=(kf == FT - 1),
                                )
                    for i, mq in enumerate(mqs):
                        for ns in range(D // 512):
                            sl = slice(ns * 512, (ns + 1) * 512)
                            nc.vector.tensor_add(
                                out=out_t[:, mq, sl], in0=ps2[i * 2 + ns][:],
                                in1=h2_s[:, mq, sl],
                            )
                        layer_norm(
                            fln, out_t[:, mq, :], out_t[:, mq, :], g3_b, lb3_b
                        )
                        nc.sync.dma_start(
                            out=out_d.rearrange("(t p) d -> p t d", p=128)[:, mq, :],
                            in_=out_t[:, mq, :],
                        )

    _split_sync_waits(nc)
    return nc


_NC_CACHE = None


def _get_program():
    global _NC_CACHE
    if _NC_CACHE is None:
        _NC_CACHE = _build_program()
    return _NC_CACHE


# ----------------------------------------------------------------------------
# host wrapper
# ----------------------------------------------------------------------------

def _col_interleave(v, nt):
    """[n] f32 -> [128, nt] where col j holds v[j*128:(j+1)*128]."""
    return np.ascontiguousarray(
        np.asarray(v, np.float32).reshape(nt, 128).T
    )


def kernel(**inputs):
    x = np.asarray(inputs["cur_input"], np.float32)          # [B, S, D]
    cls = np.asarray(inputs["classVector"], np.float32)      # [B, 1, 10]
    mask = np.asarray(inputs["attn_mask"])                   # [S, S] bool

    bf = lambda a: np.ascontiguousarray(np.asarray(a, np.float32)).astype(NP_BF16)
    f8 = lambda a: np.ascontiguousarray(np.asarray(a, np.float32)).astype(NP_F8)
    f32 = lambda a: np.ascontiguousarray(np.asarray(a, np.float32))
    row = lambda v: f32(np.asarray(v, np.float32).reshape(1, -1))

    # cross-attn over a single key collapses to a query-independent row:
    # r = ((cls @ ce_w + ce_b) @ ca_wv + ca_bv) @ ca_wo + ca_bo  (per batch)
    cv = cls[:, 0, :] @ np.asarray(inputs["ce_w"], np.float32) + np.asarray(
        inputs["ce_b"], np.float32
    )
    vcv = cv @ np.asarray(inputs["ca_wv"], np.float32) + np.asarray(
        inputs["ca_bv"], np.float32
    )
    r_rows = vcv @ np.asarray(inputs["ca_wo"], np.float32) + np.asarray(
        inputs["ca_bo"], np.float32
    )  # [B, D]
    lb1 = np.asarray(inputs["ln1_b"], np.float32)

    shared = dict(
        wq8=f8(inputs["sa_wq"]),
        wk8=f8(inputs["sa_wk"]),
        wv8=f8(inputs["sa_wv"]),
        wo=bf(inputs["sa_wo"]),
        # gamma2 folded into w1 rows; lb2 folded into b1 (and into cb_r below)
        w1=bf(np.asarray(inputs["ff_w1"], np.float32)
              * np.asarray(inputs["ln2_g"], np.float32)[:, None]),
        w2=bf(inputs["ff_w2"]),
        bq_c=_col_interleave(inputs["sa_bq"], DT),
        bk_c=_col_interleave(inputs["sa_bk"], DT),
        b1_c=_col_interleave(
            np.asarray(inputs["ff_b1"], np.float32)
            + np.asarray(inputs["ln2_b"], np.float32)
            @ np.asarray(inputs["ff_w1"], np.float32), FT),
        bv_r=row(inputs["sa_bv"]),
        cb_r=row(np.asarray(inputs["ff_b2"], np.float32)
                 + np.asarray(inputs["ln2_b"], np.float32)),
        g1_r=row(inputs["ln1_g"]),
        g2_r=row(inputs["ln2_g"]),
        g3_r=row(inputs["ln3_g"]),
        lb3_r=row(inputs["ln3_b"]),
    )

    mT = mask.T.astype(np.float32)  # [S key, S query]
    bo = np.asarray(inputs["sa_bo"], np.float32)
    xT8_b = [
        f8(x[b].T.reshape(DT, 128, S).transpose(1, 0, 2)) for b in range(B)
    ]
    in_maps = []
    for c in range(NCORES):
        b, q0 = c // (NCORES // B), (c % (NCORES // B)) * QS
        mTc = mT[:, q0 : q0 + QS].reshape(ST, 128, QS).transpose(1, 0, 2)
        in_maps.append(
            dict(
                shared,
                xT8=xT8_b[b],
                xqT8=np.ascontiguousarray(xT8_b[b][:, :, q0 : q0 + QS]),
                xrows=f32(x[b, q0 : q0 + QS, :] + bo),
                maskT=bf(mTc),
                lb1r_r=row(lb1 + r_rows[b]),
            )
        )

    res = run_bass_kernel_spmd(_get_program(), in_maps, list(range(NCORES)))
    out = np.empty((B, S, D), np.float32)
    for c in range(NCORES):
        b, q0 = c // (NCORES // B), (c % (NCORES // B)) * QS
        out[b, q0 : q0 + QS] = res.results[c]["out"]
    return out


# revision 33
# speedup vs baseline: 1.3262x; 1.0880x over previous
"""Trainium2 Bass kernel for nn_Block_86672440033530 (sparse_attention).

Transformer block: masked self-attention + AddNorm, class-vector cross-attn
(collapses to a host-computed broadcast row since Sk=1) + AddNorm, FFN + AddNorm.

Sharding: 8 cores = 2 batches x 4 query-blocks of 512 rows. K/V projections are
sharded across each 4-core batch group (each core projects its own 512 keys)
and exchanged with two AllGather collectives; everything else is row-local.

Precision: Q/K/V projections, QK, and AV run in fp8(e4m3) — DoubleRow fp8
matmuls for the projections and AV (K=256 per step), row-tiled K=64 matmul
pairs for QK (two heads concurrently in the PE array). The attention output is
a tiny fraction of the residual stream here, so fp8 error washes out. O-proj
and the FFN stay bf16 (they carry ~half the stream); PSUM always fp32. The
softmax denominator comes from a ones-column appended to V (fused into the AV
matmul) and is divided out via a DRAM-bounce partition broadcast.
"""
import contextlib
import ctypes
import sys
import types

import numpy as np

if "/opt/trn_rl_repo" not in sys.path:
    sys.path.insert(0, "/opt/trn_rl_repo")

import ml_dtypes  # noqa: E402
import concourse.bass as bass  # noqa: E402
import concourse.mybir as mybir  # noqa: E402
import concourse.tile as tile  # noqa: E402
from concourse.bass_utils import run_bass_kernel_spmd  # noqa: E402
from concourse.masks import make_identity  # noqa: E402

BF16 = mybir.dt.bfloat16
F32 = mybir.dt.float32
F8 = mybir.dt.float8e4
NP_BF16 = ml_dtypes.bfloat16
NP_F8 = ml_dtypes.float8_e4m3

B, S, D, H, DFF = 2, 2048, 1024, 16, 4096
HD = D // H                      # 64
NCORES = 8
QS = S // (NCORES // B)          # 512 query rows per core
QT = QS // 128                   # 4 query tiles per core
DT = D // 128                    # 8 d-blocks
ST = S // 128                    # 16 key tiles
FT = DFF // 128                  # 32 dff tiles
EPS = 1e-5
GROUPS = [[0, 1, 2, 3], [4, 5, 6, 7]]
DR = mybir.MatmulPerfMode.DoubleRow


def _install_ntff_shim():
    """The axon image lacks antenv.axon_hooks; register the NTFF profile hook
    via ctypes so run_bass_kernel_spmd(trace=True) works. Harmless if unused."""
    try:
        import antenv
    except ImportError:
        return
    if "antenv.axon_hooks" in sys.modules:
        return

    def _make_hook(so_path):
        try:
            lib = ctypes.CDLL(so_path)
        except OSError:
            return None
        if not hasattr(lib, "axon_start_nrt_profile"):
            return None
        lib.axon_start_nrt_profile.argtypes = [
            ctypes.POINTER(ctypes.c_int64),
            ctypes.c_size_t,
        ]
        lib.axon_start_nrt_profile.restype = ctypes.c_int64
        lib.axon_stop_nrt_profile.argtypes = [ctypes.c_char_p]
        lib.axon_stop_nrt_profile.restype = ctypes.c_int64

        @contextlib.contextmanager
        def _hook(output_dir, device_ids):
            import jax

            jax.devices()
            if device_ids:
                ids = (ctypes.c_int64 * len(device_ids))(*device_ids)
                rc = lib.axon_start_nrt_profile(ids, len(device_ids))
            else:
                rc = lib.axon_start_nrt_profile(None, 0)
            if rc != 0:
                raise RuntimeError(f"axon_start_nrt_profile rc={rc}")
            try:
                yield
            finally:
                n = lib.axon_stop_nrt_profile(str(output_dir).encode())
                print(f"profile: {n} file(s) -> {output_dir}", file=sys.stderr)

        return _hook

    m = types.ModuleType("antenv.axon_hooks")
    m._hook = _make_hook("/opt/axon/libaxon_pjrt.so")
    m.set_axon_ntff_profile_hook = lambda h: setattr(m, "_hook", h)
    m.get_axon_ntff_profile_hook = lambda: m._hook
    sys.modules["antenv.axon_hooks"] = m
    import antenv

    antenv.axon_hooks = m


_install_ntff_shim()


def _split_sync_waits(nc, limit=1):
    """This walrus build accepts at most one sync-wait command per
    instruction; move excess waits onto same-engine NoOps placed before."""
    for func in nc.m.functions:
        for bb in func.blocks:
            out = []
            for ins in bb.instructions:
                si = getattr(ins, "sync_info", None)
                waits = list(si.on_wait) if (si is not None and si.on_wait) else []
                if len(waits) > limit:
                    keep, move = waits[:limit], waits[limit:]
                    for i in range(0, len(move), limit):
                        out.append(
                            mybir.InstNoOp(
                                name=f"{ins.name}-wsplit{i}",
                                sync_info=mybir.SyncInfo(
                                    on_wait=move[i : i + limit], on_update=[]
                                ),
                                bass_nofuse=True,
                                engine=ins.engine,
                            )
                        )
                    si.on_wait = keep
                out.append(ins)
            bb.instructions[:] = out


# ----------------------------------------------------------------------------
# device program (SPMD; identical on all 8 cores, per-core data differs)
# ----------------------------------------------------------------------------

def _build_program():
    nc = bass.Bass()

    def din(name, shape, dt):
        return nc.dram_tensor(name, list(shape), dt, kind="ExternalInput")

    # per-core tensors
    xT8 = din("xT8", [128, DT, S], F8)          # x[b].T full (d-major), fp8
    xqT8 = din("xqT8", [128, DT, QS], F8)       # own rows of x[b].T, fp8
    xrows = din("xrows", [QS, D], F32)          # own rows + bo (residual, host)
    maskT = din("maskT", [128, ST, QS], BF16)   # mask.T own q cols {0,1}
    # weights
    wq8 = din("wq8", [D, D], F8)
    wk8 = din("wk8", [D, D], F8)
    wv8 = din("wv8", [D, D], F8)
    wo = din("wo", [D, D], BF16)
    w1 = din("w1", [D, DFF], BF16)              # gamma2 folded into rows
    w2 = din("w2", [DFF, D], BF16)
    # f32 bias/ln vectors: column-interleaved [128, n] or rows [1, n]
    bq_c = din("bq_c", [128, DT], F32)
    bk_c = din("bk_c", [128, DT], F32)
    b1_c = din("b1_c", [128, FT], F32)
    bv_r = din("bv_r", [1, D], F32)
    cb_r = din("cb_r", [1, D], F32)             # lb2 + b2 (host)
    g1_r = din("g1_r", [1, D], F32)
    lb1r_r = din("lb1r_r", [1, D], F32)         # ln1_b + cross-attn row (host)
    g2_r = din("g2_r", [1, D], F32)
    g3_r = din("g3_r", [1, D], F32)
    lb3_r = din("lb3_r", [1, D], F32)

    out_d = nc.dram_tensor("out", [QS, D], F32, kind="ExternalOutput")

    Exp = mybir.ActivationFunctionType.Exp
    Relu = mybir.ActivationFunctionType.Relu
    Sqrt = mybir.ActivationFunctionType.Sqrt
    ADD = mybir.AluOpType.add
    SUB = mybir.AluOpType.subtract
    MUL = mybir.AluOpType.mult

    with tile.TileContext(nc) as tc, contextlib.ExitStack() as ctx:
        # -------- whole-kernel residents (small) ---------------------------
        res = ctx.enter_context(tc.tile_pool(name="res", bufs=1))

        ident = res.tile([128, 128], BF16)
        make_identity(nc, ident)
        eps_t = res.tile([128, 1], F32)
        nc.vector.memset(eps_t[:], EPS)

        def bcast_load(pool, src_row, n, tag, eng=None):
            t = pool.tile([128, n], F32, tag=tag)
            e = eng if eng is not None else nc.sync
            e.dma_start(out=t[:], in_=src_row[0:1, :].broadcast_to((128, n)))
            return t

        def layer_norm(pool, dst, src, g_b=None, lb_b=None):
            """dst = LN_freedim(src) [* g] [+ b] for [128, D] f32 views."""
            stats = pool.tile([128, 2, 6], F32, tag="lnst")
            mv = pool.tile([128, 2], F32, tag="lnmv")
            for sg in range(2):
                nc.vector.bn_stats(
                    out=stats[:, sg, :], in_=src[:, sg * 512 : (sg + 1) * 512]
                )
            nc.vector.bn_aggr(out=mv[:], in_=stats[:])
            rstd = pool.tile([128, 1], F32, tag="lnrs")
            nc.scalar.activation(
                out=rstd[:], in_=mv[:, 1:2], func=Sqrt, bias=eps_t[:]
            )
            nc.vector.reciprocal(out=rstd[:], in_=rstd[:])
            nc.vector.tensor_scalar(
                out=dst[:], in0=src[:], scalar1=mv[:, 0:1], scalar2=rstd[:],
                op0=SUB, op1=MUL,
            )
            if g_b is not None:
                nc.vector.tensor_mul(out=dst[:], in0=dst[:], in1=g_b[:])
            if lb_b is not None:
                nc.vector.tensor_add(out=dst[:], in0=dst[:], in1=lb_b[:])

        # pool for phase-O/F tiles prefetched during P/A
        of = ctx.enter_context(tc.tile_pool(name="of", bufs=1))
        h2_s = of.tile([128, QT, D], F32)
        h2T_s = of.tile([128, DT, QS], BF16)
        w1_pre = of.tile([128, DT, 1024], BF16)  # FF1 group 0, loaded in O
        # xr/wo/oT live P..O only (released before FFN to make room for w2)
        po = tc.alloc_tile_pool(name="po", bufs=1)
        xr_s = po.tile([128, QT, D], F32)
        wo_t = po.tile([128, DT, D], BF16)
        oT_s = po.tile([128, DT, QS], BF16)      # attention output (transposed)

        # -------- phases P+A share the big attention residents -------------
        with tc.tile_pool(name="pa", bufs=1) as pa, \
             tc.tile_pool(name="pad", bufs=1, space="DRAM") as pad:
            kT_s = pa.tile([128, 4, DT, QS], F8)        # K.T d-major, by rank
            vp_s = pa.tile([128, ST, H, HD + 1], F8)    # V natural + ones col
            qT_s = pa.tile([128, DT, QS], F8)           # Q.T (d-major)
            maskT_s = pa.tile([128, ST, QS], BF16)

            agk_in = pad.tile([1, 128 * DT * QS], F8, tag="agki")
            agk_out = pad.tile([4, 128 * DT * QS], F8, tag="agko")

            # ---- phase P: K own-shard + AllGather; V/Q local --------------
            with tc.tile_pool(name="pph", bufs=1) as pp, \
                 tc.tile_pool(name="pps", bufs=4, space="PSUM") as pps:
                x_s = pp.tile([128, DT, S], F8)
                xq_s = pp.tile([128, DT, QS], F8)
                wk_s = pp.tile([128, DT, D], F8, tag="w8", bufs=2, name="wk_s")
                wv_s = pp.tile([128, DT, D], F8, tag="w8", bufs=2, name="wv_s")
                wq_s = pp.tile([128, DT, D], F8, tag="w8", bufs=2, name="wq_s")
                nc.gpsimd.dma_start(
                    out=wk_s[:], in_=wk8.rearrange("(a p) n -> p a n", p=128)
                )
                for k in range(DT):
                    nc.sync.dma_start(out=xq_s[:, k, :], in_=xqT8[:, k, :])
                bk_s = pp.tile([128, DT], F32)
                bq_s = pp.tile([128, DT], F32)
                nc.sync.dma_start(out=bk_s[:], in_=bk_c[:])
                nc.sync.dma_start(out=bq_s[:], in_=bq_c[:])
                nc.gpsimd.dma_start(
                    out=wv_s[:], in_=wv8.rearrange("(a p) n -> p a n", p=128)
                )
                nc.gpsimd.dma_start(
                    out=wq_s[:], in_=wq8.rearrange("(a p) n -> p a n", p=128)
                )
                for k in range(DT):
                    nc.sync.dma_start(out=x_s[:, k, :], in_=xT8[:, k, :])
                bv_b = bcast_load(pp, bv_r, D, "bvb", eng=nc.scalar)
                # phase-O/A prefetches ride the scalar issue queue (idle in P)
                nc.scalar.dma_start(
                    out=maskT_s[:],
                    in_=maskT.rearrange("p a q -> p (a q)")
                    .rearrange("p (a q) -> p a q", a=ST),
                )
                nc.scalar.dma_start(
                    out=xr_s[:], in_=xrows.rearrange("(t p) d -> p t d", p=128)
                )
                nc.scalar.dma_start(
                    out=wo_t[:], in_=wo.rearrange("(a p) n -> p a n", p=128)
                )

                # K for own keys only (DoubleRow fp8, K=256 per step)
                k_own = pp.tile([128, DT, QS], F8)
                xq = xq_s[:]
                for m in range(DT):
                    ps = pps.tile([128, QS], F32, tag="pj")
                    for kp in range(DT // 2):
                        nc.tensor.matmul(
                            ps[:],
                            wk_s[:, 2 * kp : 2 * kp + 2, m * 128 : (m + 1) * 128],
                            xq[:, 2 * kp : 2 * kp + 2, :],
                            start=(kp == 0), stop=(kp == DT // 2 - 1),
                            perf_mode=DR,
                        )
                    nc.vector.tensor_scalar(
                        out=k_own[:, m, :], in0=ps[:],
                        scalar1=bk_s[:, m : m + 1], scalar2=None, op0=ADD,
                    )
                nc.sync.dma_start(
                    out=agk_in[:].rearrange("o (p a q) -> (o p) a q", p=128, a=DT),
                    in_=k_own[:],
                )
                nc.gpsimd.collective_compute(
                    "AllGather", mybir.AluOpType.bypass,
                    ins=[agk_in[:]], outs=[agk_out[:]], replica_groups=GROUPS,
                )

                # V for ALL keys (natural layout + ones col), fp8, local
                nc.vector.memset(vp_s[:, :, :, HD : HD + 1], 1.0)
                for st in range(ST):
                    pss = [
                        pps.tile([128, 512], F32, tag="pj", name=f"vps{st}_{c}")
                        for c in range(2)
                    ]
                    for kp in range(DT // 2):
                        for c in range(2):
                            nc.tensor.matmul(
                                pss[c][:],
                                x_s[:, 2 * kp : 2 * kp + 2,
                                    st * 128 : (st + 1) * 128],
                                wv_s[:, 2 * kp : 2 * kp + 2,
                                     c * 512 : (c + 1) * 512],
                                start=(kp == 0), stop=(kp == DT // 2 - 1),
                                perf_mode=DR,
                            )
                    for c in range(2):
                        nc.vector.tensor_add(
                            out=vp_s[:, st, c * 8 : (c + 1) * 8, 0:HD],
                            in0=pss[c][:].rearrange("p (h e) -> p h e", e=HD),
                            in1=bv_b[:, c * 512 : (c + 1) * 512].rearrange(
                                "p (h e) -> p h e", e=HD
                            ),
                        )
                # Q (own rows)
                for m in range(DT):
                    ps = pps.tile([128, QS], F32, tag="pj")
                    for kp in range(DT // 2):
                        nc.tensor.matmul(
                            ps[:],
                            wq_s[:, 2 * kp : 2 * kp + 2, m * 128 : (m + 1) * 128],
                            xq[:, 2 * kp : 2 * kp + 2, :],
                            start=(kp == 0), stop=(kp == DT // 2 - 1),
                            perf_mode=DR,
                        )
                    nc.vector.tensor_scalar(
                        out=qT_s[:, m, :], in0=ps[:],
                        scalar1=bq_s[:, m : m + 1], scalar2=None, op0=ADD,
                    )

                # AllGather return
                for r in range(4):
                    nc.sync.dma_start(
                        out=kT_s[:, r, :, :],
                        in_=agk_out[r : r + 1, :].rearrange(
                            "o (p a q) -> (o p) a q", p=128, a=DT
                        ),
                    )

            # ---- phase A: attention ---------------------------------------
            with tc.tile_pool(name="aph", bufs=1) as apl, \
                 tc.tile_pool(name="aqk", bufs=3, space="PSUM") as aqk, \
                 tc.tile_pool(name="avps", bufs=1, space="PSUM") as avps, \
                 tc.tile_pool(name="adr", bufs=1, space="DRAM") as adr:
                den_d = [
                    adr.tile([4, QS], F32, tag="dend", name=f"den_d{_b}", bufs=4)
                    for _b in range(4)
                ]
                den_d2 = [
                    adr.tile([4, QS], F32, tag="dend2", name=f"den_d2{_b}", bufs=4)
                    for _b in range(4)
                ]

                def normalize_batch(b, tail=False):
                    # reciprocal of this batch's 4 den rows ([32,64] shape for
                    # lane parallelism), then broadcast and scale 2 oT tiles
                    den_sb = apl.tile([32, 64], F32, tag="densb", name=f"densb{b}")
                    flat = den_d[b].rearrange("a q -> (a q)")
                    nc.sync.dma_start(
                        out=den_sb[:], in_=flat.rearrange("(p f) -> p f", f=64)
                    )
                    nc.vector.reciprocal(out=den_sb[:], in_=den_sb[:])
                    flat2 = den_d2[b].rearrange("a q -> (a q)")
                    nc.sync.dma_start(
                        out=flat2.rearrange("(p f) -> p f", f=64), in_=den_sb[:]
                    )
                    rb2 = apl.tile([128, 2, QS], F32, tag="rb2", name=f"rb2{b}")
                    dv = den_d2[b].rearrange("(a e) q -> e a q", e=2)
                    nc.sync.dma_start(
                        out=rb2[0:64, :, :],
                        in_=dv[0:1, :, :].broadcast_to((64, 2, QS)),
                    )
                    nc.sync.dma_start(
                        out=rb2[64:128, :, :],
                        in_=dv[1:2, :, :].broadcast_to((64, 2, QS)),
                    )
                    eng = nc.vector if tail else nc.gpsimd
                    for tt in range(2):
                        t = b * 2 + tt
                        eng.tensor_mul(
                            out=oT_s[:, t, :], in0=oT_s[:, t, :],
                            in1=rb2[:, tt, :],
                        )

                for m in range(DT):
                    avs = [
                        avps.tile([HD + 1, QS], F32, tag=f"av{_e}",
                                  name=f"av{m}_{_e}")
                        for _e in range(2)
                    ]
                    for jp in range(ST // 2):
                        pt = apl.tile([128, 2, 2, QS], F8, tag="pt", bufs=2)
                        pe = apl.tile([128, 2, 2, QS], BF16, tag="pe", bufs=2)
                        for jj in range(2):
                            j = jp * 2 + jj
                            qk = aqk.tile([128, 2, 512], F32, tag="qk")
                            nc.tensor.matmul(
                                qk[:, 0, :],
                                kT_s[0:64, j // 4, m, (j % 4) * 128 : (j % 4) * 128 + 128],
                                qT_s[0:64, m, :],
                                start=True, stop=True,
                            )
                            nc.tensor.matmul(
                                qk[:, 1, :],
                                kT_s[64:128, j // 4, m, (j % 4) * 128 : (j % 4) * 128 + 128],
                                qT_s[64:128, m, :],
                                start=True, stop=True,
                            )
                            nc.scalar.activation(
                                pe[:, jj, :, :], qk[:], Exp, scale=0.125
                            )
                            nc.vector.tensor_mul(
                                out=pt[:, jj, :, :], in0=pe[:, jj, :, :],
                                in1=maskT_s[:, j, :][:, None, :].broadcast_to(
                                    (128, 2, QS)
                                ),
                            )
                        for e in range(2):
                            nc.tensor.matmul(
                                avs[e][:],
                                vp_s[:, 2 * jp : 2 * jp + 2, 2 * m + e, :],
                                pt[:, :, e, :],
                                start=(jp == 0), stop=(jp == ST // 2 - 1),
                                perf_mode=DR,
                            )
                    # stash denominator rows (via DRAM); evict unnormalized
                    for e in range(2):
                        h = 2 * m + e
                        dr_row = apl.tile([1, QS], F32, tag="dr", bufs=2)
                        nc.vector.tensor_copy(out=dr_row[:], in_=avs[e][HD : HD + 1, :])
                        nc.sync.dma_start(
                            out=den_d[h // 4][h % 4 : h % 4 + 1, :], in_=dr_row[:]
                        )
                        nc.vector.tensor_copy(
                            out=oT_s[e * 64 : e * 64 + 64, m, :], in_=avs[e][0:HD, :]
                        )
                    if m == 1:
                        normalize_batch(0)
                    elif m == 3:
                        normalize_batch(1)
                    elif m == 5:
                        normalize_batch(2)
                normalize_batch(3, tail=True)

        # FF1 group-0 weights load during phase O (scalar queue is idle here)
        nc.scalar.dma_start(
            out=w1_pre[:],
            in_=w1.rearrange("(a p) n -> p a n", p=128)[:, :, 0:1024],
        )

        # -------- phase O: out-proj, AddNorm, LN2, transpose ---------------
        with tc.tile_pool(name="oph", bufs=1) as op, \
             tc.tile_pool(name="ops", bufs=4, space="PSUM") as ops, \
             tc.tile_pool(name="otps", bufs=2, space="PSUM") as otps, \
             tc.tile_pool(name="oln", bufs=4) as oln:
            g1_b = bcast_load(op, g1_r, D, "g1b")
            lb1_b = bcast_load(op, lb1r_r, D, "lb1b")   # ln1_b + r (host)

            h_s = op.tile([128, QT, D], F32)
            for mq in range(QT):
                pss = [
                    ops.tile([128, 512], F32, tag="op", name=f"ops{mq}_{ns}")
                    for ns in range(D // 512)
                ]
                for k in range(DT):
                    for ns in range(D // 512):
                        nc.tensor.matmul(
                            pss[ns][:],
                            oT_s[:, k, mq * 128 : (mq + 1) * 128],
                            wo_t[:, k, ns * 512 : (ns + 1) * 512],
                            start=(k == 0), stop=(k == DT - 1),
                        )
                for ns in range(D // 512):
                    sl = slice(ns * 512, (ns + 1) * 512)
                    nc.vector.tensor_add(
                        out=h_s[:, mq, sl], in0=pss[ns][:], in1=xr_s[:, mq, sl]
                    )
                layer_norm(oln, h_s[:, mq, :], h_s[:, mq, :], g1_b, lb1_b)
                # h2n = pure-normalized LN2; gamma2/beta2 folded into w1/b1
                # (host) and into the FF2-tail residual
                layer_norm(oln, h2_s[:, mq, :], h_s[:, mq, :])
                h2bf = oln.tile([128, D], BF16, tag="h2bf")
                nc.scalar.copy(out=h2bf[:], in_=h2_s[:, mq, :])
                for t in range(DT):
                    tp = otps.tile([128, 128], BF16, tag="tp")
                    nc.tensor.transpose(
                        tp[:], h2bf[:, t * 128 : (t + 1) * 128], ident[:]
                    )
                    nc.scalar.copy(
                        out=h2T_s[:, t, mq * 128 : (mq + 1) * 128], in_=tp[:]
                    )

        po.release()

        # -------- phase F: FFN + AddNorm -----------------------------------
        with tc.tile_pool(name="fph", bufs=1) as fp, \
             tc.tile_pool(name="fw1", bufs=2) as fw1, \
             tc.tile_pool(name="fln", bufs=4) as fln:
            # preload all of w2 while FF1 computes (split across queues so
            # multiple DMA engines stream in parallel)
            w2_s = fp.tile([128, FT, D], BF16)
            w2r = w2.rearrange("(a p) n -> p a n", p=128)
            for g in range(8):
                eng = nc.scalar if g % 2 == 0 else nc.gpsimd
                eng.dma_start(
                    out=w2_s[:, g * 4 : (g + 1) * 4, :],
                    in_=w2r[:, g * 4 : (g + 1) * 4, :],
                )
            b1_s = fp.tile([128, FT], F32)
            nc.sync.dma_start(out=b1_s[:], in_=b1_c[:])
            g2_b = bcast_load(fp, g2_r, D, "g2b")
            cb_b = bcast_load(fp, cb_r, D, "cbb")   # lb2 + b2 (host)
            g3_b = bcast_load(fp, g3_r, D, "g3b")
            lb3_b = bcast_load(fp, lb3_r, D, "lb3b")
            # h2full = h2n * g2 + (lb2 + b2), off the critical path
            for mq in range(QT):
                nc.gpsimd.tensor_mul(
                    out=h2_s[:, mq, :], in0=h2_s[:, mq, :], in1=g2_b[:]
                )
                nc.gpsimd.tensor_add(
                    out=h2_s[:, mq, :], in0=h2_s[:, mq, :], in1=cb_b[:]
                )

            fT_s = fp.tile([128, FT, QS], BF16)
            # FF1: fT[:, mf, :] = relu(w1[:, mf].T @ h2T + b1)
            with tc.tile_pool(name="fps", bufs=3, space="PSUM") as fps:
                for mfg in range(4):
                    if mfg == 0:
                        w1_t = w1_pre
                    else:
                        w1_t = fw1.tile(
                            [128, DT, 1024], BF16, tag="w1", name=f"w1_t{mfg}"
                        )
                        nc.sync.dma_start(
                            out=w1_t[:],
                            in_=w1.rearrange("(a p) n -> p a n", p=128)[
                                :, :, mfg * 1024 : (mfg + 1) * 1024
                            ],
                        )
                    for mfl in range(8):
                        mf = mfg * 8 + mfl
                        ps = fps.tile([128, QS], F32, tag="f1")
                        for k in range(DT):
                            nc.tensor.matmul(
                                ps[:],
                                w1_t[:, k, mfl * 128 : (mfl + 1) * 128],
                                h2T_s[:, k, :],
                                start=(k == 0), stop=(k == DT - 1),
                            )
                        nc.scalar.activation(
                            out=fT_s[:, mf, :], in_=ps[:], func=Relu,
                            bias=b1_s[:, mf : mf + 1],
                        )

            # FF2 in two mq-halves so the first half's LN3/output
            # overlaps the second half's matmuls
            out_t = fp.tile([128, QT, D], F32)
            with tc.tile_pool(name="f2ps", bufs=8, space="PSUM") as f2ps:
                for half in range(2):
                    mqs = (0, 1) if half == 0 else (2, 3)
                    ps2 = [
                        f2ps.tile([128, 512], F32, tag="f2",
                                  name=f"ps2_{half}_{_i}")
                        for _i in range(2 * len(mqs))
                    ]
                    for kf in range(FT):
                        for i, mq in enumerate(mqs):
                            for ns in range(D // 512):
                                nc.tensor.matmul(
                                    ps2[i * 2 + ns][:],
                                    fT_s[:, kf, mq * 128 : (mq + 1) * 128],
                                    w2_s[:, kf, ns * 512 : (ns + 1) * 512],
                                    start=(kf == 0), stop=(kf == FT - 1),
                                )
                    for i, mq in enumerate(mqs):
                        for ns in range(D // 512):
                            sl = slice(ns * 512, (ns + 1) * 512)
                            nc.vector.tensor_add(
                                out=out_t[:, mq, sl], in0=ps2[i * 2 + ns][:],
                                in1=h2_s[:, mq, sl],
                            )
                        layer_norm(
                            fln, out_t[:, mq, :], out_t[:, mq, :], g3_b, lb3_b
                        )
                        nc.sync.dma_start(
                            out=out_d.rearrange("(t p) d -> p t d", p=128)[:, mq, :],
                            in_=out_t[:, mq, :],
                        )

    _split_sync_waits(nc)
    return nc


_NC_CACHE = None


def _get_program():
    global _NC_CACHE
    if _NC_CACHE is None:
        _NC_CACHE = _build_program()
    return _NC_CACHE


# ----------------------------------------------------------------------------
# host wrapper
# ----------------------------------------------------------------------------

def _col_interleave(v, nt):
    """[n] f32 -> [128, nt] where col j holds v[j*128:(j+1)*128]."""
    return np.ascontiguousarray(
        np.asarray(v, np.float32).reshape(nt, 128).T
    )


def kernel(**inputs):
    x = np.asarray(inputs["cur_input"], np.float32)          # [B, S, D]
    cls = np.asarray(inputs["classVector"], np.float32)      # [B, 1, 10]
    mask = np.asarray(inputs["attn_mask"])                   # [S, S] bool

    bf = lambda a: np.ascontiguousarray(np.asarray(a, np.float32)).astype(NP_BF16)
    f8 = lambda a: np.ascontiguousarray(np.asarray(a, np.float32)).astype(NP_F8)
    f32 = lambda a: np.ascontiguousarray(np.asarray(a, np.float32))
    row = lambda v: f32(np.asarray(v, np.float32).reshape(1, -1))

    # cross-attn over a single key collapses to a query-independent row:
    # r = ((cls @ ce_w + ce_b) @ ca_wv + ca_bv) @ ca_wo + ca_bo  (per batch)
    cv = cls[:, 0, :] @ np.asarray(inputs["ce_w"], np.float32) + np.asarray(
        inputs["ce_b"], np.float32
    )
    vcv = cv @ np.asarray(inputs["ca_wv"], np.float32) + np.asarray(
        inputs["ca_bv"], np.float32
    )
    r_rows = vcv @ np.asarray(inputs["ca_wo"], np.float32) + np.asarray(
        inputs["ca_bo"], np.float32
    )  # [B, D]
    lb1 = np.asarray(inputs["ln1_b"], np.float32)

    shared = dict(
        wq8=f8(inputs["sa_wq"]),
        wk8=f8(inputs["sa_wk"]),
        wv8=f8(inputs["sa_wv"]),
        wo=bf(inputs["sa_wo"]),
        # gamma2 folded into w1 rows; lb2 folded into b1 (and into cb_r below)
        w1=bf(np.asarray(inputs["ff_w1"], np.float32)
              * np.asarray(inputs["ln2_g"], np.float32)[:, None]),
        w2=bf(inputs["ff_w2"]),
        bq_c=_col_interleave(inputs["sa_bq"], DT),
        bk_c=_col_interleave(inputs["sa_bk"], DT),
        b1_c=_col_interleave(
            np.asarray(inputs["ff_b1"], np.float32)
            + np.asarray(inputs["ln2_b"], np.float32)
            @ np.asarray(inputs["ff_w1"], np.float32), FT),
        bv_r=row(inputs["sa_bv"]),
        cb_r=row(np.asarray(inputs["ff_b2"], np.float32)
                 + np.asarray(inputs["ln2_b"], np.float32)),
        g1_r=row(inputs["ln1_g"]),
        g2_r=row(inputs["ln2_g"]),
        g3_r=row(inputs["ln3_g"]),
        lb3_r=row(inputs["ln3_b"]),
    )

    mT = mask.T.astype(np.float32)  # [S key, S query]
    bo = np.asarray(inputs["sa_bo"], np.float32)
    xT8_b = [
        f8(x[b].T.reshape(DT, 128, S).transpose(1, 0, 2)) for b in range(B)
    ]
    in_maps = []
    for c in range(NCORES):
        b, q0 = c // (NCORES // B), (c % (NCORES // B)) * QS
        mTc = mT[:, q0 : q0 + QS].reshape(ST, 128, QS).transpose(1, 0, 2)
        in_maps.append(
            dict(
                shared,
                xT8=xT8_b[b],
                xqT8=np.ascontiguousarray(xT8_b[b][:, :, q0 : q0 + QS]),
                xrows=f32(x[b, q0 : q0 + QS, :] + bo),
                maskT=bf(mTc),
                lb1r_r=row(lb1 + r_rows[b]),
            )
        )

    res = run_bass_kernel_spmd(_get_program(), in_maps, list(range(NCORES)))
    out = np.empty((B, S, D), np.float32)
    for c in range(NCORES):
        b, q0 = c // (NCORES // B), (c % (NCORES // B)) * QS
        out[b, q0 : q0 + QS] = res.results[c]["out"]
    return out
